# revision 47
# baseline (speedup 1.0000x reference)
"""DeepseekV2 MLA attention on 8 Trainium2 NeuronCores (Bass/Tile), v7.

Token-sharded front end (bf16 q_a/q_b/kv_a for accuracy); the 576-row kv
latent (normalized kv_a + roped k_pe) is AllGathered early — the Pool queue
carries ONLY the collectives so the AllGather fires as soon as the latent
is staged (~13us).  q_b outputs are exchanged per head in fp8 as 256 rows
per dest [qn(128) | qpe(64) | qpe_resid(64)]: the residual rides the
otherwise-wasted pad half of the DoubleRow pe-chunk and cancels the fp8
quantization of q_pe.  Scores run as fp8e4 DoubleRow matmuls
(lhsT=(kn | kpe,kpe-copy), rhs=(qn | qpe,resid), 2x128 contraction per
instruction at 0.5 cyc/row); the causal mask is added in PSUM by a DR
(ident,0)x(maskd,0) matmul with exact fp8 constants {0,-448}; the softmax
SCALE is applied inside the exp activation.  exp runs once per ki-pair on
[128,2,512] PSUM tiles.  v/e/attnV/w_o stay bf16 (fp8 v measurably breaks
the 2e-2 gate); z = ones^T . esum with DVE pair-sums.  Row-parallel w_o;
host sums the 8 bf16 partials in fp32.
"""

import numpy as np
import ml_dtypes

import concourse.bass as bass
import concourse.bacc as bacc
import concourse.mybir as mybir
import concourse.tile as tile
from concourse import bass_utils

T = 2048
HID = 2048
H = 16
DN = 128
DR = 64
DV = 128
DQK = DN + DR
QLR = 1536
KVLR = 512
THETA = 10000.0
EPS = 1e-6
SCALE = DQK ** -0.5

NCORES = 8
HPC = H // NCORES
LATR = KVLR + DR          # 576 rows of exchanged kv latent

F32 = mybir.dt.float32
BF = mybir.dt.bfloat16
F8 = mybir.dt.float8e4
BF_NP = ml_dtypes.bfloat16
F8_NP = ml_dtypes.float8_e4m3
DRMODE = mybir.MatmulPerfMode.DoubleRow

KT = HID // 128           # 16 contraction strips over hidden
QMT = QLR // 128          # 12
KVMT = KVLR // 128        # 4
NB = T // 512             # 4 query blocks
TBT = T // 128            # 16 token blocks
TSH = T // NCORES         # 256 tokens per shard

QCH = 3 * 128             # 384 q_b output rows per dest (qn0,qn1,pe-pair)
QEX = 256                 # exchanged rows per dest per head
MASKV = -240.0            # max-finite of IEEE e4m3; -240*SCALE = -17.3 in exp


def build_bass():
    nc = bacc.Bacc(
        "TRN2",
        target_bir_lowering=False,
        debug=False,
        enable_asserts=False,
        num_devices=NCORES,
    )

    hs_sh = nc.dram_tensor("hs_sh", [HID, TSH], BF, kind="ExternalInput").ap()
    hs_full = nc.dram_tensor("hs_full", [KT * 128, T], BF, kind="ExternalInput").ap()
    wqa = nc.dram_tensor("wqa", [QMT * 128, KT * 128], BF, kind="ExternalInput").ap()
    wkva = nc.dram_tensor("wkva", [KVMT * 128, KT * 128], BF, kind="ExternalInput").ap()
    wkpe = nc.dram_tensor("wkpe", [128, KT * DR], BF, kind="ExternalInput").ap()
    wqb = nc.dram_tensor("wqb", [NCORES * 128, QMT * QCH], BF, kind="ExternalInput").ap()
    wkvb = nc.dram_tensor("wkvb", [128, KVMT * 4 * 128], BF, kind="ExternalInput").ap()
    wo = nc.dram_tensor("wo", [HPC * DV, HID], BF, kind="ExternalInput").ap()
    cosf2 = nc.dram_tensor("cosf2", [128, TSH], BF, kind="ExternalInput").ap()
    sinf2 = nc.dram_tensor("sinf2", [128, TSH], BF, kind="ExternalInput").ap()
    cosk = nc.dram_tensor("cosk", [DR, T], BF, kind="ExternalInput").ap()
    sink = nc.dram_tensor("sink", [DR, T], BF, kind="ExternalInput").ap()
    perm128 = nc.dram_tensor("perm128", [128, 128], BF, kind="ExternalInput").ap()
    selswap = nc.dram_tensor("selswap", [128, 128], BF, kind="ExternalInput").ap()
    identz = nc.dram_tensor("identz", [128, 2 * 128], F8, kind="ExternalInput").ap()
    maskdz = nc.dram_tensor("maskdz", [128, 4 * 1024], F8, kind="ExternalInput").ap()
    ones = nc.dram_tensor("ones", [128, 128], BF, kind="ExternalInput").ap()
    out = nc.dram_tensor("out", [T, HID], BF, kind="ExternalOutput").ap()

    with tile.TileContext(nc) as tc:
        _kernel_body(nc, tc, hs_sh, hs_full, wqa, wkva, wkpe, wqb, wkvb, wo,
                     cosf2, sinf2, cosk, sink, perm128, selswap, identz,
                     maskdz, ones, out)

    nc.compile()
    return nc


def _kernel_body(nc, tc, hs_sh, hs_full, wqa, wkva, wkpe, wqb, wkvb, wo,
                 cosf2, sinf2, cosk, sink, perm128, selswap, identz, maskdz,
                 ones, out):
    from contextlib import ExitStack

    ctx = ExitStack()
    with ctx:
        dram = ctx.enter_context(tc.tile_pool(name="dram", bufs=1, space="DRAM"))
        contrib_kv = dram.tile([KVLR, TSH], BF)
        a2a_kv = dram.tile([NCORES * KVLR, TSH], BF)
        contrib_qh = [dram.tile([NCORES * QEX, TSH], F8, name=f"cq{h}")
                      for h in range(HPC)]
        a2a_qh = [dram.tile([NCORES * QEX, TSH], F8, name=f"aq{h}")
                  for h in range(HPC)]

        persist = ctx.enter_context(tc.tile_pool(name="persist", bufs=1))
        # persist DMAs ride the Act queue behind hs2/wkva1/wkva3 so SP is
        # free for the rest of the AllGather-critical path.
        ones_t = persist.tile([128, 128], BF, tag="ones")
        cos_t = persist.tile([128, TSH], BF, tag="cos")
        sin_t = persist.tile([128, TSH], BF, tag="sin")
        identz_t = persist.tile([128, 2, 128], F8, tag="identz")
        maskdz_t = persist.tile([128, 4, 2, 512], F8, tag="maskdz")
        wkvb_t = persist.tile([128, KVMT, 4 * 128], BF, tag="wkvb")
        perm_t = persist.tile([128, 128], BF, tag="perm")
        selswap_t = persist.tile([128, 128], BF, tag="selswap")
        wo_t = [persist.tile([128, HID], BF, tag=f"wo{h}", name=f"wo{h}")
                for h in range(HPC)]
        wq_t = []
        for d in range(NCORES):
            wq_t.append(persist.tile([128, QMT * QCH], BF, tag=f"wq{d}",
                                     name=f"wq{d}"))
        ones_col = ones_t[:, 0:1]
        ones_row = ones_t[0:1, :]

        cosk_t = persist.tile([DR, T], BF, tag="cosk")
        sink_t = persist.tile([DR, T], BF, tag="sink")

        def _persist_early():
            # needed by the kv-latent critical path (rope, rsqrt broadcast)
            # and the q_b rope (perm/selswap)
            nc.scalar.dma_start(out=ones_t, in_=ones)
            nc.scalar.dma_start(out=cos_t, in_=cosf2)
            nc.scalar.dma_start(out=sin_t, in_=sinf2)
            nc.scalar.dma_start(out=perm_t, in_=perm128)
            nc.scalar.dma_start(out=selswap_t, in_=selswap)
            # preload the Sqrt act-func set off the critical path
            actwarm = persist.tile([1, 8], F32, tag="actwarm")
            nc.scalar.activation(actwarm, ones_t[0:1, 0:8],
                                 mybir.ActivationFunctionType.Sqrt)

        def _persist_late():
            # emitted after the AllGather is issued, on SP behind the wq
            # stream — none of these are needed before ~60us
            nc.sync.dma_start(
                out=identz_t, in_=identz.rearrange("p (c k) -> p c k", c=2))
            nc.sync.dma_start(
                out=maskdz_t,
                in_=maskdz.rearrange("p (s c f) -> p s c f", s=4, c=2))
            nc.sync.dma_start(
                out=wkvb_t, in_=wkvb.rearrange("p (s c) -> p s c", s=KVMT))
            nc.sync.dma_start(out=cosk_t, in_=cosk)
            nc.sync.dma_start(out=sink_t, in_=sink)
            for h in range(HPC):
                nc.sync.dma_start(out=wo_t[h],
                                  in_=wo[h * DV:(h + 1) * DV, :])

        pmid = ctx.enter_context(tc.tile_pool(name="pmid", bufs=1))

        # ---- Phase A: latents on own shard --------------------------------
        # Pool queue carries ONLY the collectives: the AllGather is issued
        # first and fires as soon as its contrib DMAs (on the DVE queue)
        # complete.  Weight DMAs ride SP/Act around the critical path.
        with tc.tile_pool(name="pa", bufs=1) as pa, \
             tc.tile_pool(name="psa", bufs=1, space="PSUM") as psa:
            # warm-up: memset feeds dummy matmuls that ramp the PE p-state
            # while the first DMAs land (cost model: 3us of continuous PE
            # execution reaches full clock).
            warm_t = pa.tile([128, 256], BF, tag="warm")
            nc.vector.memset(warm_t, 1.0)
            warm_ps = psa.tile([128, TSH], F32, tag="pq", bufs=3,
                               name="warm_ps")
            for i in range(44):
                nc.tensor.matmul(warm_ps[0:1, 0:64], lhsT=warm_t[:, 0:1],
                                 rhs=warm_t[:, 0:64], start=True, stop=True,
                                 skip_group_check=True)
            # hs/wkva split across SP and Act so the four kv_a strips land
            # by ~5us; everything else queues behind them.
            wkva4_t = pa.tile([128, KVMT, KT * 128], BF, tag="wkva4")
            hs_t = pa.tile([128, KT, TSH], BF, tag="hst")
            nc.sync.dma_start(out=wkva4_t[:, 0, :], in_=wkva[0:128, :])
            nc.scalar.dma_start(
                out=hs_t[:, 0:KT // 2, :],
                in_=hs_sh[0:HID // 2].rearrange("(k p) t -> p k t", p=128))
            nc.sync.dma_start(
                out=hs_t[:, KT // 2:, :],
                in_=hs_sh[HID // 2:].rearrange("(k p) t -> p k t", p=128))
            nc.scalar.dma_start(out=wkva4_t[:, 1, :], in_=wkva[128:256, :])
            nc.sync.dma_start(out=wkva4_t[:, 2, :], in_=wkva[256:384, :])
            nc.scalar.dma_start(out=wkva4_t[:, 3, :], in_=wkva[384:, :])
            wkpe_t = persist.tile([128, KT * DR], BF, tag="wkpe")
            nc.sync.dma_start(out=wkpe_t, in_=wkpe)
            _persist_early()
            hst = [hs_t[:, k, :] for k in range(KT)]
            wkva_t = [wkva4_t[:, m, :] for m in range(KVMT)]

            def rsqrt_bc(z_psum, n, tag):
                # rsqrt(z/n + eps) = sqrt(n / (z + n*eps)): DVE add+recip,
                # one Act Sqrt hop (Act queue is kept clear of big DMAs here)
                tmp = pa.tile([1, TSH], F32, tag="rsq_tmp", bufs=2)
                nc.vector.tensor_scalar_add(tmp, z_psum, n * EPS)
                nc.vector.reciprocal(tmp, tmp)
                srow = pa.tile([1, TSH], BF, tag=tag + "r", name=tag + "r")
                nc.scalar.activation(srow, tmp,
                                     mybir.ActivationFunctionType.Sqrt,
                                     scale=float(n))
                b_ps = psa.tile([128, TSH], F32, tag="bc", bufs=1,
                                name="b_ps")
                nc.tensor.matmul(b_ps, lhsT=ones_row, rhs=srow,
                                 start=True, stop=True)
                bc = pmid.tile([128, TSH], BF, tag=tag, name=tag)
                nc.vector.tensor_copy(bc, b_ps)
                return bc

            zkv = psa.tile([1, TSH], F32, tag="zkv")
            kv_raw = []   # bf16 un-normalized latent strips
            for m in range(KVMT):
                pq = psa.tile([128, TSH], F32, tag="pq", bufs=3)
                for k in range(KT):
                    nc.tensor.matmul(pq, lhsT=wkva_t[m][:, k * 128:(k + 1) * 128],
                                     rhs=hst[k],
                                     start=(k == 0), stop=(k == KT - 1))
                st = pa.tile([128, TSH], BF, tag=f"kvr{m}", name=f"kvr{m}")
                nc.vector.tensor_copy(st, pq)
                kv_raw.append(st)
                sq = pa.tile([128, TSH], BF, tag="sq", bufs=2)
                nc.vector.tensor_tensor(sq, st, st, op=mybir.AluOpType.mult)
                nc.tensor.matmul(zkv, lhsT=ones_col, rhs=sq,
                                 start=(m == 0), stop=(m == KVMT - 1))

            skv_bc = rsqrt_bc(zkv, KVLR, "skvbc")
            # normalized latent staged contiguously for one contrib DMA
            kvstage = pa.tile([128, KVMT, TSH], BF, tag="kvstage")
            for m in range(KVMT):
                nc.vector.tensor_tensor(kvstage[:, m, :], kv_raw[m], skv_bc,
                                        op=mybir.AluOpType.mult)

            # contrib DMA rides the Pool queue itself — idle, dedicated, and
            # immediately ahead of the AllGather, so no other ready work can
            # steal its slot.  (k_pe is computed replicated, not exchanged.)
            nc.gpsimd.dma_start(
                out=contrib_kv.rearrange("(g p) t -> p g t", p=128),
                in_=kvstage)
            nc.gpsimd.collective_compute(
                "AllGather", mybir.AluOpType.bypass,
                replica_groups=[list(range(NCORES))],
                ins=[contrib_kv], outs=[a2a_kv])
            _persist_late()

            # q_b weights on SP behind the front; q_a strips on Act
            for d in range(NCORES):
                nc.sync.dma_start(out=wq_t[d],
                                  in_=wqb[d * 128:(d + 1) * 128, :])

            # q latent
            zq = psa.tile([1, TSH], F32, tag="zq")
            q_raw = []
            for m in range(QMT):
                wt = pa.tile([128, KT * 128], BF, tag="wqa", bufs=4)
                nc.scalar.dma_start(out=wt, in_=wqa[m * 128:(m + 1) * 128, :])
                pq = psa.tile([128, TSH], F32, tag="pq", bufs=3)
                for k in range(KT):
                    nc.tensor.matmul(pq, lhsT=wt[:, k * 128:(k + 1) * 128],
                                     rhs=hst[k],
                                     start=(k == 0), stop=(k == KT - 1))
                st = pmid.tile([128, TSH], BF, tag=f"qr{m}", name=f"qr{m}")
                nc.vector.tensor_copy(st, pq)
                q_raw.append(st)
                sq = pa.tile([128, TSH], BF, tag="sq", bufs=2)
                nc.vector.tensor_tensor(sq, st, st, op=mybir.AluOpType.mult)
                nc.tensor.matmul(zq, lhsT=ones_col, rhs=sq,
                                 start=(m == 0), stop=(m == QMT - 1))
            sq_bc = rsqrt_bc(zq, QLR, "sqbc")
            # preload the Exp act-func set well before the first real exp
            actwarm2 = pa.tile([1, 8], F32, tag="actwarm2")
            nc.scalar.activation(actwarm2, ones_t[0:1, 0:8],
                                 mybir.ActivationFunctionType.Exp)
            qan = []
            for m in range(QMT):
                qq_ = pmid.tile([128, TSH], BF, tag=f"qan{m}", name=f"qan{m}")
                nc.vector.tensor_tensor(qq_, q_raw[m], sq_bc,
                                        op=mybir.AluOpType.mult)
                qan.append(qq_)

        bcp = ctx.enter_context(tc.tile_pool(name="bcp", bufs=1))
        # kk: DoubleRow score lhsT per head: [dims, shard, half, slot, 128tok]
        # slots per token block: [kn_h0 | kpe-pair | kn_h1 | kpe-pair]
        # head h uses slots (2h, 2h+1); slot 1 == slot 3 = [kpe ; kpe-copy].
        kk = bcp.tile([128, NCORES, 2, 4, 128], F8, tag="kk", name="kk")
        kpe8 = bcp.tile([DR, NCORES, 2, 128], F8, tag="kpe8", name="kpe8")

        # ---- q_b for all dests + per-head exchange ------------------------
        # head-0 AllToAll goes first so head-0 attention can overlap the
        # head-1 AllToAll.  Exchange rows per dest: [qn128 | qpe64 | resid64].
        # The replicated-k_pe work shares these pools so nothing serializes
        # on pool open/close.
        with tc.tile_pool(name="pw", bufs=1) as pw, \
             tc.tile_pool(name="psw", bufs=1, space="PSUM") as psw:
            st_qn = [pw.tile([128, NCORES, TSH], F8, tag=f"stqn{h}",
                             name=f"stqn{h}") for h in range(HPC)]
            st_pe = [pw.tile([128, NCORES, TSH], F8, tag=f"stpe{h}",
                             name=f"stpe{h}") for h in range(HPC)]
            cos64 = cos_t[0:DR, :]
            sin64 = sin_t[0:DR, :]
            for d in range(NCORES):
                wq = wq_t[d]
                accq = []
                for mt in range(3):
                    a = psw.tile([128, TSH], F32, tag="acc", bufs=3,
                                 name=f"accq{mt}")
                    accq.append(a)
                for k in range(QMT):
                    for mt in range(3):
                        nc.tensor.matmul(
                            accq[mt],
                            lhsT=wq[:, k * QCH + mt * 128:k * QCH + (mt + 1) * 128],
                            rhs=qan[k],
                            start=(k == 0), stop=(k == QMT - 1))
                for hh in range(HPC):
                    nc.vector.tensor_copy(st_qn[hh][:, d, :], accq[hh])
                # q_pe rope, heads split to base-0 64-row tiles
                qraw = pw.tile([128, TSH], BF, tag="qraw", bufs=2)
                nc.vector.tensor_copy(qraw, accq[2])
                rope3 = psw.tile([DR, 3, TSH], F32, tag="rope3", bufs=1)
                sw0, raw1, sw1 = rope3[:, 0, :], rope3[:, 1, :], rope3[:, 2, :]
                nc.tensor.matmul(sw0, lhsT=perm_t[:, 0:DR], rhs=qraw,
                                 start=True, stop=True)
                nc.tensor.matmul(raw1, lhsT=selswap_t[:, 0:DR], rhs=qraw,
                                 start=True, stop=True)
                nc.tensor.matmul(sw1, lhsT=selswap_t[:, DR:2 * DR], rhs=qraw,
                                 start=True, stop=True)
                for hh in range(HPC):
                    r1 = pw.tile([DR, TSH], BF, tag=f"r1_{hh}", bufs=2)
                    nc.vector.tensor_tensor(
                        r1, qraw[0:DR, :] if hh == 0 else raw1, cos64,
                        op=mybir.AluOpType.mult)
                    r2 = pw.tile([DR, TSH], BF, tag=f"r2_{hh}", bufs=2)
                    nc.vector.tensor_tensor(
                        r2, sw0 if hh == 0 else sw1, sin64,
                        op=mybir.AluOpType.mult)
                    pe_bf = pw.tile([DR, TSH], BF, tag=f"pebf{hh}", bufs=2)
                    nc.vector.tensor_tensor(pe_bf, r1, r2,
                                            op=mybir.AluOpType.add)
                    nc.vector.tensor_copy(st_pe[hh][0:DR, d, :], pe_bf)
                    # residual of the fp8 cast (mixed-dtype subtract)
                    nc.vector.tensor_tensor(st_pe[hh][DR:128, d, :],
                                            pe_bf, st_pe[hh][0:DR, d, :],
                                            op=mybir.AluOpType.subtract)
            for h in range(HPC):
                nc.sync.dma_start(
                    out=contrib_qh[h].rearrange("(d c) t -> c d t",
                                                d=NCORES)[0:128],
                    in_=st_qn[h])
                nc.sync.dma_start(
                    out=contrib_qh[h].rearrange("(d c) t -> c d t",
                                                d=NCORES)[128:QEX],
                    in_=st_pe[h])
                nc.gpsimd.collective_compute(
                    "AllToAll", mybir.AluOpType.bypass,
                    replica_groups=[list(range(NCORES))],
                    ins=[contrib_qh[h]], outs=[a2a_qh[h]])

            # ---- replicated k_pe over all tokens --------------------------
            # Real PE work that fills the AllGather window (and keeps the
            # p-state warm): k_pe = w_kpe^T . hs for all 2048 tokens, roped.
            kpel_all = pw.tile([DR, NCORES, 2, 128], BF, tag="kpelall")
            for qt_ in range(4):
                tslh = slice(qt_ * 512, (qt_ + 1) * 512)
                kpe_ps = psw.tile([DR, 512], F32, tag="kpeps", bufs=1)
                for k in range(KT):
                    hstrip = pw.tile([128, 512], BF, tag="hsf", bufs=14)
                    eng = nc.sync if k % 2 == 0 else nc.scalar
                    eng.dma_start(out=hstrip,
                                  in_=hs_full[k * 128:(k + 1) * 128, tslh])
                    nc.tensor.matmul(kpe_ps,
                                     lhsT=wkpe_t[:, k * DR:(k + 1) * DR],
                                     rhs=hstrip,
                                     start=(k == 0), stop=(k == KT - 1))
                kpe_rawh = pw.tile([DR, 512], BF, tag="kpraw", bufs=2)
                nc.vector.tensor_copy(kpe_rawh, kpe_ps)
                swh_ps = psw.tile([DR, 512], F32, tag="swk", bufs=1)
                nc.tensor.matmul(swh_ps, lhsT=perm_t[0:DR, 0:DR],
                                 rhs=kpe_rawh, start=True, stop=True)
                rt1 = pw.tile([DR, 512], BF, tag="rt1k", bufs=2)
                nc.vector.tensor_tensor(rt1, kpe_rawh, cosk_t[:, tslh],
                                        op=mybir.AluOpType.mult)
                rt2 = pw.tile([DR, 512], BF, tag="rt2k", bufs=2)
                nc.vector.tensor_tensor(rt2, swh_ps, sink_t[:, tslh],
                                        op=mybir.AluOpType.mult)
                nc.vector.tensor_tensor(
                    kpel_all[:, 2 * qt_:2 * qt_ + 2, :, :], rt1, rt2,
                    op=mybir.AluOpType.add)
            nc.vector.tensor_copy(kpe8, kpel_all)
            for pr in range(2):
                dst = kk[pr * DR:(pr + 1) * DR]
                eng = nc.sync if pr == 0 else nc.scalar
                eng.dma_start(out=dst[:, :, :, 1, :], in_=kpe8)
                eng.dma_start(out=dst[:, :, :, 3, :], in_=kpe8)
            # keep the PE p-state warm until the AllGather lands
            wwps = psw.tile([DR, 512], F32, tag="kpeps", bufs=1, name="wwps")
            for i in range(100):
                nc.tensor.matmul(wwps[0:1, 0:256], lhsT=kpe8[:, 0, 0, 0:1],
                                 rhs=kpe8[:, 0, :, :], start=True, stop=True,
                                 skip_group_check=True)


        # ---- Phase B: expand k_nope / v for own heads over all tokens -----
        kvan = []      # latent strips, all tokens [128, 8, 256]
        for r in range(KVMT):
            kt_ = bcp.tile([128, NCORES, TSH], BF, tag=f"kvan{r}",
                           name=f"kvan{r}")
            eng = nc.sync if r % 2 == 0 else nc.scalar
            eng.dma_start(
                out=kt_,
                in_=a2a_kv.rearrange("(s r) t -> r s t", s=NCORES)
                            [r * 128:(r + 1) * 128])
            kvan.append(kt_)

        def tok512(tile3, c):
            # 512-token chunk c of a [*, 8, 256] tile
            return tile3[:, 2 * c:2 * c + 2, :]

        def tok128(tile3, tb):
            half = (tb % 2) * 128
            return tile3[:, tb // 2, half:half + 128]

        vt = [None] * TBT   # per 128-token block [128, HPC*DV] token-major v
        with tc.tile_pool(name="pb", bufs=1) as pb, \
             tc.tile_pool(name="psb", bufs=1, space="PSUM") as psb:
            for h in range(HPC):
                for c in range(4):
                    acck = psb.tile([128, 512], F32, tag="acck", bufs=2)
                    for s in range(KVMT):
                        nc.tensor.matmul(
                            acck, lhsT=wkvb_t[:, s, h * DN:(h + 1) * DN],
                            rhs=tok512(kvan[s], c),
                            start=(s == 0), stop=(s == KVMT - 1))
                    nc.vector.tensor_copy(
                        kk[:, 2 * c:2 * c + 2, :, 2 * h, :]
                        .rearrange("p s j c -> p (s j) c"),
                        acck.rearrange("p (f c) -> p f c", c=128))
            for tb in range(TBT):
                accv = psb.tile([128, HPC * DV], F32, tag="accv", bufs=3)
                for s in range(KVMT):
                    nc.tensor.matmul(
                        accv, lhsT=tok128(kvan[s], tb),
                        rhs=wkvb_t[:, s, 2 * DN:2 * DN + HPC * DV],
                        start=(s == 0), stop=(s == KVMT - 1))
                vt[tb] = bcp.tile([128, HPC * DV], BF, tag=f"v{tb}",
                                  name=f"v{tb}")
                nc.vector.tensor_copy(vt[tb], accv)

        # q tiles for own heads, all tokens: [dims, chunk, shard, 256]
        # chunk0 = qn (128 dims); chunk1 = [qpe 64 | qpe_resid 64].
        qq = []
        for h in range(HPC):
            qt = bcp.tile([128, 2, NCORES, TSH], F8, tag=f"qq{h}",
                          name=f"qq{h}")
            src = a2a_qh[h].rearrange("(s c) t -> c s t", s=NCORES)
            engs = ((nc.sync, nc.scalar) if h == 0 else (nc.gpsimd, nc.sync))
            for sh in range(2):
                ssl = slice(sh * 4, sh * 4 + 4)
                qeng = engs[sh]
                qeng.dma_start(out=qt[:, 0, ssl, :], in_=src[0:128, ssl])
                qeng.dma_start(out=qt[0:DR, 1, ssl, :],
                               in_=src[128:128 + DR, ssl])
                qeng.dma_start(out=qt[DR:128, 1, ssl, :],
                               in_=src[128 + DR:QEX, ssl])
            qq.append(qt)

        # ---- Attention + output projection --------------------------------
        with tc.tile_pool(name="pc", bufs=1) as pc, \
             tc.tile_pool(name="psc", bufs=1, space="PSUM") as psc:
            attn_n = [[None] * NB for _ in range(HPC)]
            for h in range(HPC):
                for qj in reversed(range(NB)):
                    npair = 2 * qj + 2
                    q_rhs = qq[h][:, :, 2 * qj:2 * qj + 2, :]
                    attn_ps = psc.tile([128, 512], F32, tag="attn", bufs=2)
                    esums = []   # tree-reduced pair sums (short dep chains)
                    for p in range(npair):
                        s_pair = psc.tile([128, 2, 512], F32, tag="s", bufs=2)
                        for i in range(2):
                            ki = 2 * p + i
                            diag = ki >= 4 * qj
                            nc.tensor.matmul(
                                s_pair[:, i, :],
                                lhsT=kk[:, ki // 2, ki % 2,
                                        2 * h:2 * h + 2, :],
                                rhs=q_rhs,
                                start=True, stop=not diag,
                                perf_mode=DRMODE)
                            if diag:
                                sub = ki - 4 * qj
                                nc.tensor.matmul(
                                    s_pair[:, i, :],
                                    lhsT=identz_t,
                                    rhs=maskdz_t[:, sub],
                                    start=False, stop=True,
                                    perf_mode=DRMODE)
                        e_pair = pc.tile([128, 2, 512], BF, tag="e", bufs=6)
                        nc.scalar.activation(e_pair, s_pair,
                                             mybir.ActivationFunctionType.Exp,
                                             scale=SCALE)
                        for i in range(2):
                            ki = 2 * p + i
                            nc.tensor.matmul(
                                attn_ps,
                                lhsT=vt[ki][:, h * DV:(h + 1) * DV],
                                rhs=e_pair[:, i, :],
                                start=(p == 0 and i == 0),
                                stop=(p == npair - 1 and i == 1))
                        et = pc.tile([128, 512], BF, tag="etree", bufs=8)
                        nc.vector.tensor_tensor(
                            et, e_pair[:, 0, :], e_pair[:, 1, :],
                            op=mybir.AluOpType.add)
                        esums.append(et)
                        # opportunistic tree combine of completed pairs
                        while len(esums) >= 2 and p < npair - 1:
                            b = esums.pop()
                            a = esums.pop()
                            et2 = pc.tile([128, 512], BF, tag="etree",
                                          bufs=8)
                            nc.vector.tensor_tensor(
                                et2, a, b, op=mybir.AluOpType.add)
                            esums.append(et2)
                            break
                    while len(esums) > 1:
                        b = esums.pop()
                        a = esums.pop()
                        et2 = pc.tile([128, 512], BF, tag="etree", bufs=8)
                        nc.vector.tensor_tensor(et2, a, b,
                                                op=mybir.AluOpType.add)
                        esums.append(et2)
                    esum = esums[0]
                    # z = colsum(esum); broadcast 1/z via ones matmul
                    zrow_ps = psc.tile([128, 512], F32, tag="o", bufs=2,
                                       name="zrow_ps")
                    nc.tensor.matmul(zrow_ps[0:1, :], lhsT=ones_col,
                                     rhs=esum, start=True, stop=True)
                    zr = pc.tile([1, 512], BF, tag="zr", bufs=2)
                    with nc.allow_low_precision(reason="bf16 softmax"):
                        nc.vector.tensor_copy(zr, zrow_ps[0:1, :])
                    zb_ps = psc.tile([128, 512], F32, tag="o", bufs=2,
                                     name="zb_ps")
                    nc.tensor.matmul(zb_ps, lhsT=ones_row, rhs=zr,
                                     start=True, stop=True)
                    rzb = pc.tile([128, 512], BF, tag="rzb", bufs=2)
                    with nc.allow_low_precision(reason="bf16 softmax"):
                        nc.vector.reciprocal(rzb, zb_ps)
                    attn_n[h][qj] = bcp.tile([128, 512], BF,
                                             tag=f"attn{h}_{qj}",
                                             name=f"attn{h}_{qj}")
                    nc.vector.tensor_tensor(attn_n[h][qj], attn_ps, rzb,
                                            op=mybir.AluOpType.mult)

                    if h == HPC - 1:
                        # both heads' attn_n for this query block are ready
                        for tt in range(4):
                            tb = qj * 4 + tt
                            tsl = slice(tt * 128, (tt + 1) * 128)
                            last = (qj == 0 and tt == 3)  # last in exec order
                            o_row = pc.tile([128, HID], BF, tag="orow",
                                            bufs=2)
                            for hb in range(NB):
                                o_ps = psc.tile([128, 512], F32, tag="o",
                                                bufs=2)
                                for hh in range(HPC):
                                    nc.tensor.matmul(
                                        o_ps,
                                        lhsT=attn_n[hh][qj][:, tsl],
                                        rhs=wo_t[hh][:, hb * 512:(hb + 1) * 512],
                                        start=(hh == 0),
                                        stop=(hh == HPC - 1),
                                    )
                                osl = o_row[:, hb * 512:(hb + 1) * 512]
                                if hb % 2 == 0:
                                    nc.vector.tensor_copy(osl, o_ps)
                                else:
                                    nc.gpsimd.tensor_copy(osl, o_ps)
                                if last:
                                    weng = nc.sync if hb % 2 == 0 else nc.scalar
                                    weng.dma_start(
                                        out=out[tb * 128:(tb + 1) * 128,
                                                hb * 512:(hb + 1) * 512],
                                        in_=osl)
                            if not last:
                                nc.sync.dma_start(
                                    out=out[tb * 128:(tb + 1) * 128, :],
                                    in_=o_row)


_NC_CACHE = {}


def _get_nc():
    if "nc" not in _NC_CACHE:
        _NC_CACHE["nc"] = build_bass()
    return _NC_CACHE["nc"]


def make_in_maps(positions, hidden_states, w_q_a, q_a_ln_w, w_q_b, w_kv_a,
                 kv_a_ln_w, w_kv_b, w_o):
    positions = np.asarray(positions)
    hidden_states = np.asarray(hidden_states, dtype=np.float32)
    w_q_a = np.asarray(w_q_a, dtype=np.float32)
    q_a_ln_w = np.asarray(q_a_ln_w, dtype=np.float32)
    w_q_b = np.asarray(w_q_b, dtype=np.float32)
    w_kv_a = np.asarray(w_kv_a, dtype=np.float32)
    kv_a_ln_w = np.asarray(kv_a_ln_w, dtype=np.float32)
    w_kv_b = np.asarray(w_kv_b, dtype=np.float32)
    w_o = np.asarray(w_o, dtype=np.float32)

    hs_t = np.ascontiguousarray(hidden_states.T)

    order = np.concatenate([np.arange(0, DR, 2), np.arange(1, DR, 2)])

    wkva_p = w_kv_a.copy()
    wkva_p[:, KVLR:] = w_kv_a[:, KVLR:][:, order]

    inv_freq = 1.0 / (THETA ** (np.arange(0, DR, 2, dtype=np.float64) / DR))
    ang = positions.astype(np.float64)[:, None] * inv_freq[None, :]
    cosT = np.cos(ang).T.astype(np.float32)
    sinT = np.sin(ang).T.astype(np.float32)
    cosf = np.concatenate([cosT, cosT], axis=0)          # [64, T]
    sinf = np.concatenate([-sinT, sinT], axis=0)
    cosf2 = np.concatenate([cosf, cosf], axis=0)         # [128, T] two heads
    sinf2 = np.concatenate([sinf, sinf], axis=0)

    perm = np.zeros((DR, DR), dtype=np.float32)
    for i in range(DR):
        perm[i, (i + DR // 2) % DR] = 1.0
    perm128 = np.zeros((128, 128), dtype=np.float32)
    perm128[:DR, :DR] = perm
    perm128[DR:, DR:] = perm
    selswap = np.zeros((128, 128), dtype=np.float32)
    for i in range(DR):
        selswap[DR + i, i] = 1.0                      # extract h1 raw
        selswap[DR + (i + DR // 2) % DR, DR + i] = 1.0  # extract h1 swapped

    # DoubleRow mask operands: chunk0 carries the additive causal mask for
    # the 4 diagonal sub-positions ({0, -448}, exact in fp8e4m3), chunk1 = 0.
    identz = np.zeros((128, 2, 128), dtype=np.float32)
    identz[:, 0, :] = np.eye(128, dtype=np.float32)
    maskdz = np.zeros((128, 4, 2, 512), dtype=np.float32)
    p = np.arange(128)[:, None]
    f = np.arange(512)[None, :]
    for sub in range(4):
        maskdz[:, sub, 0, :] = np.where(p + 128 * sub <= f, 0.0, MASKV)

    # q_b columns per dest: [qn_h0 | qn_h1 | qpe_h0(perm) ; qpe_h1(perm)]
    # NOTE: no SCALE fold — softmax scale is applied inside the exp.
    wqb_all = np.concatenate([
        np.concatenate([
            w_q_b[:, h0 * DQK:h0 * DQK + DN],
            w_q_b[:, h1 * DQK:h1 * DQK + DN],
            w_q_b[:, h0 * DQK + DN:(h0 + 1) * DQK][:, order],
            w_q_b[:, h1 * DQK + DN:(h1 + 1) * DQK][:, order],
        ], axis=1)
        for h0, h1 in ((2 * d, 2 * d + 1) for d in range(NCORES))
    ], axis=1) * q_a_ln_w[:, None]

    def pack(w, mrows):
        Kd, Md = w.shape
        n = Md // mrows
        return np.ascontiguousarray(
            w.reshape(Kd // 128, 128, n, mrows).transpose(2, 1, 0, 3)
            .reshape(n * 128, (Kd // 128) * mrows))

    wqa_pk = pack(w_q_a, 128)
    wkva_pk = pack(wkva_p[:, :KVLR], 128)
    wkpe_pk = pack(wkva_p[:, KVLR:], DR)
    wqb_pk = pack(wqb_all, QCH)

    def bf(x):
        return np.ascontiguousarray(np.asarray(x, dtype=np.float32)).astype(BF_NP)

    def f8(x):
        return np.ascontiguousarray(np.asarray(x, dtype=np.float32)).astype(F8_NP)

    in_maps = []
    for c in range(NCORES):
        h0, h1 = HPC * c, HPC * c + 1
        # own-head kv_b columns: [kn_h0 | kn_h1 | v_h0 | v_h1], ln folded
        wkvb_own = np.concatenate([
            w_kv_b[:, h0 * (DN + DV):h0 * (DN + DV) + DN],
            w_kv_b[:, h1 * (DN + DV):h1 * (DN + DV) + DN],
            w_kv_b[:, h0 * (DN + DV) + DN:(h0 + 1) * (DN + DV)],
            w_kv_b[:, h1 * (DN + DV) + DN:(h1 + 1) * (DN + DV)],
        ], axis=1) * kv_a_ln_w[:, None]
        wkvb_pk = pack(wkvb_own, 4 * 128)
        wo_c = np.concatenate([
            w_o[h0 * DV:(h0 + 1) * DV, :],
            w_o[h1 * DV:(h1 + 1) * DV, :],
        ], axis=0)
        tsl = slice(c * TSH, (c + 1) * TSH)
        in_maps.append({
            "hs_sh": bf(hs_t[:, tsl]),
            "hs_full": bf(hs_t),
            "cosk": bf(cosf),
            "sink": bf(sinf),
            "wqa": bf(wqa_pk),
            "wkva": bf(wkva_pk),
            "wkpe": bf(wkpe_pk),
            "wqb": bf(wqb_pk),
            "wkvb": bf(wkvb_pk),
            "wo": bf(wo_c),
            "cosf2": bf(cosf2[:, tsl]),
            "sinf2": bf(sinf2[:, tsl]),
            "perm128": bf(perm128),
            "selswap": bf(selswap),
            "identz": f8(identz.reshape(128, 2 * 128)),
            "maskdz": f8(maskdz.reshape(128, 4 * 1024)),
            "ones": bf(np.ones((128, 128), dtype=np.float32)),
        })
    return in_maps


def kernel(positions, hidden_states, w_q_a, q_a_ln_w, w_q_b, w_kv_a,
           kv_a_ln_w, w_kv_b, w_o):
    nc = _get_nc()
    in_maps = make_in_maps(positions, hidden_states, w_q_a, q_a_ln_w, w_q_b,
                           w_kv_a, kv_a_ln_w, w_kv_b, w_o)
    res = bass_utils.run_bass_kernel_spmd(nc, in_maps, core_ids=list(range(NCORES)))
    acc = np.zeros((T, HID), dtype=np.float32)
    for c in range(NCORES):
        acc += np.asarray(res.results[c]["out"], dtype=np.float32)
    return acc


# revision 48
# speedup vs baseline: 1.0876x; 1.0876x over previous
"""DeepseekV2 MLA attention on 8 Trainium2 NeuronCores (Bass/Tile), v7.

Token-sharded front end (bf16 q_a/q_b/kv_a for accuracy); the 576-row kv
latent (normalized kv_a + roped k_pe) is AllGathered early — the Pool queue
carries ONLY the collectives so the AllGather fires as soon as the latent
is staged (~13us).  q_b outputs are exchanged per head in fp8 as 256 rows
per dest [qn(128) | qpe(64) | qpe_resid(64)]: the residual rides the
otherwise-wasted pad half of the DoubleRow pe-chunk and cancels the fp8
quantization of q_pe.  Scores run as fp8e4 DoubleRow matmuls
(lhsT=(kn | kpe,kpe-copy), rhs=(qn | qpe,resid), 2x128 contraction per
instruction at 0.5 cyc/row); the causal mask is added in PSUM by a DR
(ident,0)x(maskd,0) matmul with exact fp8 constants {0,-448}; the softmax
SCALE is applied inside the exp activation.  exp runs once per ki-pair on
[128,2,512] PSUM tiles.  v/e/attnV/w_o stay bf16 (fp8 v measurably breaks
the 2e-2 gate); z = ones^T . esum with DVE pair-sums.  Row-parallel w_o;
host sums the 8 bf16 partials in fp32.
"""

import numpy as np
import ml_dtypes

import concourse.bass as bass
import concourse.bacc as bacc
import concourse.mybir as mybir
import concourse.tile as tile
from concourse import bass_utils

T = 2048
HID = 2048
H = 16
DN = 128
DR = 64
DV = 128
DQK = DN + DR
QLR = 1536
KVLR = 512
THETA = 10000.0
EPS = 1e-6
SCALE = DQK ** -0.5

NCORES = 8
HPC = H // NCORES
LATR = KVLR + DR          # 576 rows of exchanged kv latent

F32 = mybir.dt.float32
BF = mybir.dt.bfloat16
F8 = mybir.dt.float8e4
BF_NP = ml_dtypes.bfloat16
F8_NP = ml_dtypes.float8_e4m3
DRMODE = mybir.MatmulPerfMode.DoubleRow

KT = HID // 128           # 16 contraction strips over hidden
QMT = QLR // 128          # 12
KVMT = KVLR // 128        # 4
NB = T // 512             # 4 query blocks
TBT = T // 128            # 16 token blocks
TSH = T // NCORES         # 256 tokens per shard

QCH = 3 * 128             # 384 q_b output rows per dest (qn0,qn1,pe-pair)
QEX = 256                 # exchanged rows per dest per head
MASKV = -240.0            # max-finite of IEEE e4m3; -240*SCALE = -17.3 in exp


def build_bass():
    nc = bacc.Bacc(
        "TRN2",
        target_bir_lowering=False,
        debug=False,
        enable_asserts=False,
        num_devices=NCORES,
    )

    hs_sh = nc.dram_tensor("hs_sh", [HID, TSH], BF, kind="ExternalInput").ap()
    hs_full = nc.dram_tensor("hs_full", [KT * 128, T], BF, kind="ExternalInput").ap()
    wqa = nc.dram_tensor("wqa", [QMT * 128, KT * 128], BF, kind="ExternalInput").ap()
    wkva = nc.dram_tensor("wkva", [KVMT * 128, KT * 128], BF, kind="ExternalInput").ap()
    wkpe = nc.dram_tensor("wkpe", [128, KT * DR], BF, kind="ExternalInput").ap()
    wqb = nc.dram_tensor("wqb", [NCORES * 128, QMT * QCH], BF, kind="ExternalInput").ap()
    wkvb = nc.dram_tensor("wkvb", [128, KVMT * 4 * 128], BF, kind="ExternalInput").ap()
    wo = nc.dram_tensor("wo", [HPC * DV, HID], BF, kind="ExternalInput").ap()
    cosf2 = nc.dram_tensor("cosf2", [128, TSH], BF, kind="ExternalInput").ap()
    sinf2 = nc.dram_tensor("sinf2", [128, TSH], BF, kind="ExternalInput").ap()
    cosk = nc.dram_tensor("cosk", [DR, T], BF, kind="ExternalInput").ap()
    sink = nc.dram_tensor("sink", [DR, T], BF, kind="ExternalInput").ap()
    perm128 = nc.dram_tensor("perm128", [128, 128], BF, kind="ExternalInput").ap()
    selswap = nc.dram_tensor("selswap", [128, 128], BF, kind="ExternalInput").ap()
    identz = nc.dram_tensor("identz", [128, 2 * 128], F8, kind="ExternalInput").ap()
    maskdz = nc.dram_tensor("maskdz", [128, 4 * 1024], F8, kind="ExternalInput").ap()
    ones = nc.dram_tensor("ones", [128, 128], BF, kind="ExternalInput").ap()
    out = nc.dram_tensor("out", [T, HID], BF, kind="ExternalOutput").ap()

    with tile.TileContext(nc) as tc:
        _kernel_body(nc, tc, hs_sh, hs_full, wqa, wkva, wkpe, wqb, wkvb, wo,
                     cosf2, sinf2, cosk, sink, perm128, selswap, identz,
                     maskdz, ones, out)

    nc.compile()
    return nc


def _kernel_body(nc, tc, hs_sh, hs_full, wqa, wkva, wkpe, wqb, wkvb, wo,
                 cosf2, sinf2, cosk, sink, perm128, selswap, identz, maskdz,
                 ones, out):
    from contextlib import ExitStack

    ctx = ExitStack()
    with ctx:
        dram = ctx.enter_context(tc.tile_pool(name="dram", bufs=1, space="DRAM"))
        contrib_kv = dram.tile([KVLR, TSH], BF)
        a2a_kv = dram.tile([NCORES * KVLR, TSH], BF)
        contrib_qh = [dram.tile([NCORES * QEX, TSH], F8, name=f"cq{h}")
                      for h in range(HPC)]
        a2a_qh = [dram.tile([NCORES * QEX, TSH], F8, name=f"aq{h}")
                  for h in range(HPC)]

        persist = ctx.enter_context(tc.tile_pool(name="persist", bufs=1))
        # persist DMAs ride the Act queue behind hs2/wkva1/wkva3 so SP is
        # free for the rest of the AllGather-critical path.
        ones_t = persist.tile([128, 128], BF, tag="ones")
        cos_t = persist.tile([128, TSH], BF, tag="cos")
        sin_t = persist.tile([128, TSH], BF, tag="sin")
        identz_t = persist.tile([128, 2, 128], F8, tag="identz")
        maskdz_t = persist.tile([128, 4, 2, 512], F8, tag="maskdz")
        wkvb_t = persist.tile([128, KVMT, 4 * 128], BF, tag="wkvb")
        perm_t = persist.tile([128, 128], BF, tag="perm")
        selswap_t = persist.tile([128, 128], BF, tag="selswap")
        wo_t = [persist.tile([128, HID], BF, tag=f"wo{h}", name=f"wo{h}")
                for h in range(HPC)]
        wq_t = []
        for d in range(NCORES):
            wq_t.append(persist.tile([128, QMT * QCH], BF, tag=f"wq{d}",
                                     name=f"wq{d}"))
        ones_col = ones_t[:, 0:1]
        ones_row = ones_t[0:1, :]

        cosk_t = persist.tile([DR, T], BF, tag="cosk")
        sink_t = persist.tile([DR, T], BF, tag="sink")

        def _persist_early():
            # needed by the kv-latent critical path (rope, rsqrt broadcast)
            # and the q_b rope (perm/selswap)
            nc.scalar.dma_start(out=ones_t, in_=ones)
            nc.scalar.dma_start(out=cos_t, in_=cosf2)
            nc.scalar.dma_start(out=sin_t, in_=sinf2)
            nc.scalar.dma_start(out=perm_t, in_=perm128)
            nc.scalar.dma_start(out=selswap_t, in_=selswap)
            # preload the Sqrt act-func set off the critical path
            actwarm = persist.tile([1, 8], F32, tag="actwarm")
            nc.scalar.activation(actwarm, ones_t[0:1, 0:8],
                                 mybir.ActivationFunctionType.Sqrt)

        def _persist_late():
            # emitted after the AllGather is issued, on SP behind the wq
            # stream — none of these are needed before ~60us
            nc.sync.dma_start(
                out=identz_t, in_=identz.rearrange("p (c k) -> p c k", c=2))
            nc.sync.dma_start(
                out=maskdz_t,
                in_=maskdz.rearrange("p (s c f) -> p s c f", s=4, c=2))
            nc.sync.dma_start(
                out=wkvb_t, in_=wkvb.rearrange("p (s c) -> p s c", s=KVMT))
            nc.sync.dma_start(out=cosk_t, in_=cosk)
            nc.sync.dma_start(out=sink_t, in_=sink)
            for h in range(HPC):
                nc.sync.dma_start(out=wo_t[h],
                                  in_=wo[h * DV:(h + 1) * DV, :])

        pmid = ctx.enter_context(tc.tile_pool(name="pmid", bufs=1))

        # ---- Phase A: latents on own shard --------------------------------
        # Pool queue carries ONLY the collectives: the AllGather is issued
        # first and fires as soon as its contrib DMAs (on the DVE queue)
        # complete.  Weight DMAs ride SP/Act around the critical path.
        with tc.tile_pool(name="pa", bufs=1) as pa, \
             tc.tile_pool(name="psa", bufs=1, space="PSUM") as psa:
            # warm-up: memset feeds dummy matmuls that ramp the PE p-state
            # while the first DMAs land (cost model: 3us of continuous PE
            # execution reaches full clock).
            warm_t = pa.tile([128, 256], BF, tag="warm")
            nc.vector.memset(warm_t, 1.0)
            warm_ps = psa.tile([128, TSH], F32, tag="pq", bufs=3,
                               name="warm_ps")
            for i in range(44):
                nc.tensor.matmul(warm_ps[0:1, 0:64], lhsT=warm_t[:, 0:1],
                                 rhs=warm_t[:, 0:64], start=True, stop=True,
                                 skip_group_check=True)
            # hs/wkva split across SP and Act so the four kv_a strips land
            # by ~5us; everything else queues behind them.
            wkva4_t = pa.tile([128, KVMT, KT * 128], BF, tag="wkva4")
            hs_t = pa.tile([128, KT, TSH], BF, tag="hst")
            nc.sync.dma_start(out=wkva4_t[:, 0, :], in_=wkva[0:128, :])
            nc.scalar.dma_start(
                out=hs_t[:, 0:KT // 2, :],
                in_=hs_sh[0:HID // 2].rearrange("(k p) t -> p k t", p=128))
            nc.sync.dma_start(
                out=hs_t[:, KT // 2:, :],
                in_=hs_sh[HID // 2:].rearrange("(k p) t -> p k t", p=128))
            nc.scalar.dma_start(out=wkva4_t[:, 1, :], in_=wkva[128:256, :])
            nc.sync.dma_start(out=wkva4_t[:, 2, :], in_=wkva[256:384, :])
            nc.scalar.dma_start(out=wkva4_t[:, 3, :], in_=wkva[384:, :])
            wkpe_t = persist.tile([128, KT * DR], BF, tag="wkpe")
            nc.sync.dma_start(out=wkpe_t, in_=wkpe)
            _persist_early()
            hst = [hs_t[:, k, :] for k in range(KT)]
            wkva_t = [wkva4_t[:, m, :] for m in range(KVMT)]

            def rsqrt_bc(z_psum, n, tag):
                # rsqrt(z/n + eps) = sqrt(n / (z + n*eps)): DVE add+recip,
                # one Act Sqrt hop (Act queue is kept clear of big DMAs here)
                tmp = pa.tile([1, TSH], F32, tag="rsq_tmp", bufs=2)
                nc.vector.tensor_scalar_add(tmp, z_psum, n * EPS)
                nc.vector.reciprocal(tmp, tmp)
                srow = pa.tile([1, TSH], BF, tag=tag + "r", name=tag + "r")
                nc.scalar.activation(srow, tmp,
                                     mybir.ActivationFunctionType.Sqrt,
                                     scale=float(n))
                b_ps = psa.tile([128, TSH], F32, tag="bc", bufs=1,
                                name="b_ps")
                nc.tensor.matmul(b_ps, lhsT=ones_row, rhs=srow,
                                 start=True, stop=True)
                bc = pmid.tile([128, TSH], BF, tag=tag, name=tag)
                nc.vector.tensor_copy(bc, b_ps)
                return bc

            zkv = psa.tile([1, TSH], F32, tag="zkv")
            kv_raw = []   # bf16 un-normalized latent strips
            for m in range(KVMT):
                pq = psa.tile([128, TSH], F32, tag="pq", bufs=3)
                for k in range(KT):
                    nc.tensor.matmul(pq, lhsT=wkva_t[m][:, k * 128:(k + 1) * 128],
                                     rhs=hst[k],
                                     start=(k == 0), stop=(k == KT - 1))
                st = pa.tile([128, TSH], BF, tag=f"kvr{m}", name=f"kvr{m}")
                nc.vector.tensor_copy(st, pq)
                kv_raw.append(st)
                sq = pa.tile([128, TSH], BF, tag="sq", bufs=2)
                nc.vector.tensor_tensor(sq, st, st, op=mybir.AluOpType.mult)
                nc.tensor.matmul(zkv, lhsT=ones_col, rhs=sq,
                                 start=(m == 0), stop=(m == KVMT - 1))

            skv_bc = rsqrt_bc(zkv, KVLR, "skvbc")
            # normalized latent staged contiguously for one contrib DMA
            kvstage = pa.tile([128, KVMT, TSH], BF, tag="kvstage")
            for m in range(KVMT):
                nc.vector.tensor_tensor(kvstage[:, m, :], kv_raw[m], skv_bc,
                                        op=mybir.AluOpType.mult)

            # contrib DMA rides the Pool queue itself — idle, dedicated, and
            # immediately ahead of the AllGather, so no other ready work can
            # steal its slot.  (k_pe is computed replicated, not exchanged.)
            nc.gpsimd.dma_start(
                out=contrib_kv.rearrange("(g p) t -> p g t", p=128),
                in_=kvstage)
            nc.gpsimd.collective_compute(
                "AllGather", mybir.AluOpType.bypass,
                replica_groups=[list(range(NCORES))],
                ins=[contrib_kv], outs=[a2a_kv])
            _persist_late()

            # q_b weights on SP behind the front; q_a strips on Act
            for d in range(NCORES):
                nc.sync.dma_start(out=wq_t[d],
                                  in_=wqb[d * 128:(d + 1) * 128, :])

            # q latent
            zq = psa.tile([1, TSH], F32, tag="zq")
            q_raw = []
            for m in range(QMT):
                wt = pa.tile([128, KT * 128], BF, tag="wqa", bufs=4)
                nc.scalar.dma_start(out=wt, in_=wqa[m * 128:(m + 1) * 128, :])
                pq = psa.tile([128, TSH], F32, tag="pq", bufs=3)
                for k in range(KT):
                    nc.tensor.matmul(pq, lhsT=wt[:, k * 128:(k + 1) * 128],
                                     rhs=hst[k],
                                     start=(k == 0), stop=(k == KT - 1))
                st = pmid.tile([128, TSH], BF, tag=f"qr{m}", name=f"qr{m}")
                nc.vector.tensor_copy(st, pq)
                q_raw.append(st)
                sq = pa.tile([128, TSH], BF, tag="sq", bufs=2)
                nc.vector.tensor_tensor(sq, st, st, op=mybir.AluOpType.mult)
                nc.tensor.matmul(zq, lhsT=ones_col, rhs=sq,
                                 start=(m == 0), stop=(m == QMT - 1))
            sq_bc = rsqrt_bc(zq, QLR, "sqbc")
            # preload the Exp act-func set well before the first real exp
            actwarm2 = pa.tile([1, 8], F32, tag="actwarm2")
            nc.scalar.activation(actwarm2, ones_t[0:1, 0:8],
                                 mybir.ActivationFunctionType.Exp)
            qan = []
            for m in range(QMT):
                qq_ = pmid.tile([128, TSH], BF, tag=f"qan{m}", name=f"qan{m}")
                nc.vector.tensor_tensor(qq_, q_raw[m], sq_bc,
                                        op=mybir.AluOpType.mult)
                qan.append(qq_)

        bcp = ctx.enter_context(tc.tile_pool(name="bcp", bufs=1))
        # kk: DoubleRow score lhsT per head: [dims, shard, half, slot, 128tok]
        # slots per token block: [kn_h0 | kpe-pair | kn_h1 | kpe-pair]
        # head h uses slots (2h, 2h+1); slot 1 == slot 3 = [kpe ; kpe-copy].
        kk = bcp.tile([128, NCORES, 2, 4, 128], F8, tag="kk", name="kk")
        kpe8 = bcp.tile([DR, NCORES, 2, 128], F8, tag="kpe8", name="kpe8")

        # ---- q_b for all dests + per-head exchange ------------------------
        # head-0 AllToAll goes first so head-0 attention can overlap the
        # head-1 AllToAll.  Exchange rows per dest: [qn128 | qpe64 | resid64].
        # The replicated-k_pe work shares these pools so nothing serializes
        # on pool open/close.
        with tc.tile_pool(name="pw", bufs=1) as pw, \
             tc.tile_pool(name="psw", bufs=1, space="PSUM") as psw:
            st_qn = [pw.tile([128, NCORES, TSH], F8, tag=f"stqn{h}",
                             name=f"stqn{h}") for h in range(HPC)]
            st_pe = [pw.tile([128, NCORES, TSH], F8, tag=f"stpe{h}",
                             name=f"stpe{h}") for h in range(HPC)]
            cos64 = cos_t[0:DR, :]
            sin64 = sin_t[0:DR, :]
            for d in range(NCORES):
                wq = wq_t[d]
                accq = []
                for mt in range(3):
                    a = psw.tile([128, TSH], F32, tag="acc", bufs=4,
                                 name=f"accq{mt}")
                    accq.append(a)
                for k in range(QMT):
                    for mt in range(3):
                        nc.tensor.matmul(
                            accq[mt],
                            lhsT=wq[:, k * QCH + mt * 128:k * QCH + (mt + 1) * 128],
                            rhs=qan[k],
                            start=(k == 0), stop=(k == QMT - 1))
                for hh in range(HPC):
                    nc.vector.tensor_copy(st_qn[hh][:, d, :], accq[hh])
                # q_pe rope, heads split to base-0 64-row tiles
                qraw = pw.tile([128, TSH], BF, tag="qraw", bufs=2)
                nc.vector.tensor_copy(qraw, accq[2])
                rope3 = psw.tile([DR, 3, TSH], F32, tag="rope3", bufs=1)
                sw0, raw1, sw1 = rope3[:, 0, :], rope3[:, 1, :], rope3[:, 2, :]
                nc.tensor.matmul(sw0, lhsT=perm_t[:, 0:DR], rhs=qraw,
                                 start=True, stop=True)
                nc.tensor.matmul(raw1, lhsT=selswap_t[:, 0:DR], rhs=qraw,
                                 start=True, stop=True)
                nc.tensor.matmul(sw1, lhsT=selswap_t[:, DR:2 * DR], rhs=qraw,
                                 start=True, stop=True)
                for hh in range(HPC):
                    r1 = pw.tile([DR, TSH], BF, tag=f"r1_{hh}", bufs=2)
                    nc.vector.tensor_tensor(
                        r1, qraw[0:DR, :] if hh == 0 else raw1, cos64,
                        op=mybir.AluOpType.mult)
                    r2 = pw.tile([DR, TSH], BF, tag=f"r2_{hh}", bufs=2)
                    nc.vector.tensor_tensor(
                        r2, sw0 if hh == 0 else sw1, sin64,
                        op=mybir.AluOpType.mult)
                    pe_bf = pw.tile([DR, TSH], BF, tag=f"pebf{hh}", bufs=2)
                    nc.vector.tensor_tensor(pe_bf, r1, r2,
                                            op=mybir.AluOpType.add)
                    nc.vector.tensor_copy(st_pe[hh][0:DR, d, :], pe_bf)
                    # residual of the fp8 cast (mixed-dtype subtract)
                    nc.vector.tensor_tensor(st_pe[hh][DR:128, d, :],
                                            pe_bf, st_pe[hh][0:DR, d, :],
                                            op=mybir.AluOpType.subtract)
            for h in range(HPC):
                nc.sync.dma_start(
                    out=contrib_qh[h].rearrange("(d c) t -> c d t",
                                                d=NCORES)[0:128],
                    in_=st_qn[h])
                nc.sync.dma_start(
                    out=contrib_qh[h].rearrange("(d c) t -> c d t",
                                                d=NCORES)[128:QEX],
                    in_=st_pe[h])
                nc.gpsimd.collective_compute(
                    "AllToAll", mybir.AluOpType.bypass,
                    replica_groups=[list(range(NCORES))],
                    ins=[contrib_qh[h]], outs=[a2a_qh[h]])

            # ---- replicated k_pe over all tokens --------------------------
            # Real PE work that fills the AllGather window (and keeps the
            # p-state warm): k_pe = w_kpe^T . hs for all 2048 tokens, roped.
            kpel_all = pw.tile([DR, NCORES, 2, 128], BF, tag="kpelall")
            for qt_ in range(4):
                tslh = slice(qt_ * 512, (qt_ + 1) * 512)
                kpe_ps = psw.tile([DR, 512], F32, tag="kpeps", bufs=1)
                for k in range(KT):
                    hstrip = pw.tile([128, 512], BF, tag="hsf", bufs=14)
                    eng = nc.sync if k % 2 == 0 else nc.scalar
                    eng.dma_start(out=hstrip,
                                  in_=hs_full[k * 128:(k + 1) * 128, tslh])
                    nc.tensor.matmul(kpe_ps,
                                     lhsT=wkpe_t[:, k * DR:(k + 1) * DR],
                                     rhs=hstrip,
                                     start=(k == 0), stop=(k == KT - 1))
                kpe_rawh = pw.tile([DR, 512], BF, tag="kpraw", bufs=2)
                nc.vector.tensor_copy(kpe_rawh, kpe_ps)
                swh_ps = psw.tile([DR, 512], F32, tag="swk", bufs=1)
                nc.tensor.matmul(swh_ps, lhsT=perm_t[0:DR, 0:DR],
                                 rhs=kpe_rawh, start=True, stop=True)
                rt1 = pw.tile([DR, 512], BF, tag="rt1k", bufs=2)
                nc.vector.tensor_tensor(rt1, kpe_rawh, cosk_t[:, tslh],
                                        op=mybir.AluOpType.mult)
                rt2 = pw.tile([DR, 512], BF, tag="rt2k", bufs=2)
                nc.vector.tensor_tensor(rt2, swh_ps, sink_t[:, tslh],
                                        op=mybir.AluOpType.mult)
                nc.vector.tensor_tensor(
                    kpel_all[:, 2 * qt_:2 * qt_ + 2, :, :], rt1, rt2,
                    op=mybir.AluOpType.add)
            nc.vector.tensor_copy(kpe8, kpel_all)
            for pr in range(2):
                dst = kk[pr * DR:(pr + 1) * DR]
                eng = nc.sync if pr == 0 else nc.scalar
                eng.dma_start(out=dst[:, :, :, 1, :], in_=kpe8)
                eng.dma_start(out=dst[:, :, :, 3, :], in_=kpe8)
            # keep the PE p-state warm until the AllGather lands
            wwps = psw.tile([DR, 512], F32, tag="kpeps", bufs=1, name="wwps")
            for i in range(100):
                nc.tensor.matmul(wwps[0:1, 0:256], lhsT=kpe8[:, 0, 0, 0:1],
                                 rhs=kpe8[:, 0, :, :], start=True, stop=True,
                                 skip_group_check=True)


        # ---- Phase B: expand k_nope / v for own heads over all tokens -----
        kvan = []      # latent strips, all tokens [128, 8, 256]
        for r in range(KVMT):
            kt_ = bcp.tile([128, NCORES, TSH], BF, tag=f"kvan{r}",
                           name=f"kvan{r}")
            eng = nc.sync if r % 2 == 0 else nc.scalar
            eng.dma_start(
                out=kt_,
                in_=a2a_kv.rearrange("(s r) t -> r s t", s=NCORES)
                            [r * 128:(r + 1) * 128])
            kvan.append(kt_)

        def tok512(tile3, c):
            # 512-token chunk c of a [*, 8, 256] tile
            return tile3[:, 2 * c:2 * c + 2, :]

        def tok128(tile3, tb):
            half = (tb % 2) * 128
            return tile3[:, tb // 2, half:half + 128]

        vt = [None] * TBT   # per 128-token block [128, HPC*DV] token-major v
        with tc.tile_pool(name="pb", bufs=1) as pb, \
             tc.tile_pool(name="psb", bufs=1, space="PSUM") as psb:
            for h in range(HPC):
                for c in range(4):
                    acck = psb.tile([128, 512], F32, tag="acck", bufs=2)
                    for s in range(KVMT):
                        nc.tensor.matmul(
                            acck, lhsT=wkvb_t[:, s, h * DN:(h + 1) * DN],
                            rhs=tok512(kvan[s], c),
                            start=(s == 0), stop=(s == KVMT - 1))
                    nc.vector.tensor_copy(
                        kk[:, 2 * c:2 * c + 2, :, 2 * h, :]
                        .rearrange("p s j c -> p (s j) c"),
                        acck.rearrange("p (f c) -> p f c", c=128))
            for tb in range(TBT):
                accv = psb.tile([128, HPC * DV], F32, tag="accv", bufs=3)
                for s in range(KVMT):
                    nc.tensor.matmul(
                        accv, lhsT=tok128(kvan[s], tb),
                        rhs=wkvb_t[:, s, 2 * DN:2 * DN + HPC * DV],
                        start=(s == 0), stop=(s == KVMT - 1))
                vt[tb] = bcp.tile([128, HPC * DV], BF, tag=f"v{tb}",
                                  name=f"v{tb}")
                nc.vector.tensor_copy(vt[tb], accv)

        # q tiles for own heads, all tokens: [dims, chunk, shard, 256]
        # chunk0 = qn (128 dims); chunk1 = [qpe 64 | qpe_resid 64].
        qq = []
        for h in range(HPC):
            qt = bcp.tile([128, 2, NCORES, TSH], F8, tag=f"qq{h}",
                          name=f"qq{h}")
            src = a2a_qh[h].rearrange("(s c) t -> c s t", s=NCORES)
            engs = ((nc.sync, nc.scalar) if h == 0 else (nc.gpsimd, nc.sync))
            for sh in range(2):
                ssl = slice(sh * 4, sh * 4 + 4)
                qeng = engs[sh]
                qeng.dma_start(out=qt[:, 0, ssl, :], in_=src[0:128, ssl])
                qeng.dma_start(out=qt[0:DR, 1, ssl, :],
                               in_=src[128:128 + DR, ssl])
                qeng.dma_start(out=qt[DR:128, 1, ssl, :],
                               in_=src[128 + DR:QEX, ssl])
            qq.append(qt)

        # ---- Attention + output projection --------------------------------
        with tc.tile_pool(name="pc", bufs=1) as pc, \
             tc.tile_pool(name="psc", bufs=1, space="PSUM") as psc:
            attn_n = [[None] * NB for _ in range(HPC)]
            for h in range(HPC):
                for qj in reversed(range(NB)):
                    npair = 2 * qj + 2
                    q_rhs = qq[h][:, :, 2 * qj:2 * qj + 2, :]
                    attn_ps = psc.tile([128, 512], F32, tag="attn", bufs=2)
                    esums = []   # tree-reduced pair sums (short dep chains)
                    for p in range(npair):
                        s_pair = psc.tile([128, 2, 512], F32, tag="s", bufs=2)
                        for i in range(2):
                            ki = 2 * p + i
                            diag = ki >= 4 * qj
                            nc.tensor.matmul(
                                s_pair[:, i, :],
                                lhsT=kk[:, ki // 2, ki % 2,
                                        2 * h:2 * h + 2, :],
                                rhs=q_rhs,
                                start=True, stop=not diag,
                                perf_mode=DRMODE)
                            if diag:
                                sub = ki - 4 * qj
                                nc.tensor.matmul(
                                    s_pair[:, i, :],
                                    lhsT=identz_t,
                                    rhs=maskdz_t[:, sub],
                                    start=False, stop=True,
                                    perf_mode=DRMODE)
                        e_pair = pc.tile([128, 2, 512], BF, tag="e", bufs=6)
                        nc.scalar.activation(e_pair, s_pair,
                                             mybir.ActivationFunctionType.Exp,
                                             scale=SCALE)
                        for i in range(2):
                            ki = 2 * p + i
                            nc.tensor.matmul(
                                attn_ps,
                                lhsT=vt[ki][:, h * DV:(h + 1) * DV],
                                rhs=e_pair[:, i, :],
                                start=(p == 0 and i == 0),
                                stop=(p == npair - 1 and i == 1))
                        et = pc.tile([128, 512], BF, tag="etree", bufs=8)
                        nc.vector.tensor_tensor(
                            et, e_pair[:, 0, :], e_pair[:, 1, :],
                            op=mybir.AluOpType.add)
                        esums.append(et)
                        # opportunistic tree combine of completed pairs
                        while len(esums) >= 2 and p < npair - 1:
                            b = esums.pop()
                            a = esums.pop()
                            et2 = pc.tile([128, 512], BF, tag="etree",
                                          bufs=8)
                            nc.vector.tensor_tensor(
                                et2, a, b, op=mybir.AluOpType.add)
                            esums.append(et2)
                            break
                    while len(esums) > 1:
                        b = esums.pop()
                        a = esums.pop()
                        et2 = pc.tile([128, 512], BF, tag="etree", bufs=8)
                        nc.vector.tensor_tensor(et2, a, b,
                                                op=mybir.AluOpType.add)
                        esums.append(et2)
                    esum = esums[0]
                    # z = colsum(esum); broadcast 1/z via ones matmul
                    zrow_ps = psc.tile([128, 512], F32, tag="o", bufs=2,
                                       name="zrow_ps")
                    nc.tensor.matmul(zrow_ps[0:1, :], lhsT=ones_col,
                                     rhs=esum, start=True, stop=True)
                    zr = pc.tile([1, 512], BF, tag="zr", bufs=2)
                    with nc.allow_low_precision(reason="bf16 softmax"):
                        nc.vector.tensor_copy(zr, zrow_ps[0:1, :])
                    zb_ps = psc.tile([128, 512], F32, tag="o", bufs=2,
                                     name="zb_ps")
                    nc.tensor.matmul(zb_ps, lhsT=ones_row, rhs=zr,
                                     start=True, stop=True)
                    rzb = pc.tile([128, 512], BF, tag="rzb", bufs=2)
                    with nc.allow_low_precision(reason="bf16 softmax"):
                        nc.vector.reciprocal(rzb, zb_ps)
                    attn_n[h][qj] = bcp.tile([128, 512], BF,
                                             tag=f"attn{h}_{qj}",
                                             name=f"attn{h}_{qj}")
                    nc.vector.tensor_tensor(attn_n[h][qj], attn_ps, rzb,
                                            op=mybir.AluOpType.mult)

                    if h == HPC - 1:
                        # both heads' attn_n for this query block are ready
                        for tt in range(4):
                            tb = qj * 4 + tt
                            tsl = slice(tt * 128, (tt + 1) * 128)
                            last = (qj == 0 and tt == 3)  # last in exec order
                            o_row = pc.tile([128, HID], BF, tag="orow",
                                            bufs=2)
                            for hb in range(NB):
                                o_ps = psc.tile([128, 512], F32, tag="o",
                                                bufs=2)
                                for hh in range(HPC):
                                    nc.tensor.matmul(
                                        o_ps,
                                        lhsT=attn_n[hh][qj][:, tsl],
                                        rhs=wo_t[hh][:, hb * 512:(hb + 1) * 512],
                                        start=(hh == 0),
                                        stop=(hh == HPC - 1),
                                    )
                                osl = o_row[:, hb * 512:(hb + 1) * 512]
                                if hb % 2 == 0:
                                    nc.vector.tensor_copy(osl, o_ps)
                                else:
                                    nc.gpsimd.tensor_copy(osl, o_ps)
                                if last:
                                    weng = nc.sync if hb % 2 == 0 else nc.scalar
                                    weng.dma_start(
                                        out=out[tb * 128:(tb + 1) * 128,
                                                hb * 512:(hb + 1) * 512],
                                        in_=osl)
                            if not last:
                                nc.sync.dma_start(
                                    out=out[tb * 128:(tb + 1) * 128, :],
                                    in_=o_row)


_NC_CACHE = {}


def _get_nc():
    if "nc" not in _NC_CACHE:
        _NC_CACHE["nc"] = build_bass()
    return _NC_CACHE["nc"]


def make_in_maps(positions, hidden_states, w_q_a, q_a_ln_w, w_q_b, w_kv_a,
                 kv_a_ln_w, w_kv_b, w_o):
    positions = np.asarray(positions)
    hidden_states = np.asarray(hidden_states, dtype=np.float32)
    w_q_a = np.asarray(w_q_a, dtype=np.float32)
    q_a_ln_w = np.asarray(q_a_ln_w, dtype=np.float32)
    w_q_b = np.asarray(w_q_b, dtype=np.float32)
    w_kv_a = np.asarray(w_kv_a, dtype=np.float32)
    kv_a_ln_w = np.asarray(kv_a_ln_w, dtype=np.float32)
    w_kv_b = np.asarray(w_kv_b, dtype=np.float32)
    w_o = np.asarray(w_o, dtype=np.float32)

    hs_t = np.ascontiguousarray(hidden_states.T)

    order = np.concatenate([np.arange(0, DR, 2), np.arange(1, DR, 2)])

    wkva_p = w_kv_a.copy()
    wkva_p[:, KVLR:] = w_kv_a[:, KVLR:][:, order]

    inv_freq = 1.0 / (THETA ** (np.arange(0, DR, 2, dtype=np.float64) / DR))
    ang = positions.astype(np.float64)[:, None] * inv_freq[None, :]
    cosT = np.cos(ang).T.astype(np.float32)
    sinT = np.sin(ang).T.astype(np.float32)
    cosf = np.concatenate([cosT, cosT], axis=0)          # [64, T]
    sinf = np.concatenate([-sinT, sinT], axis=0)
    cosf2 = np.concatenate([cosf, cosf], axis=0)         # [128, T] two heads
    sinf2 = np.concatenate([sinf, sinf], axis=0)

    perm = np.zeros((DR, DR), dtype=np.float32)
    for i in range(DR):
        perm[i, (i + DR // 2) % DR] = 1.0
    perm128 = np.zeros((128, 128), dtype=np.float32)
    perm128[:DR, :DR] = perm
    perm128[DR:, DR:] = perm
    selswap = np.zeros((128, 128), dtype=np.float32)
    for i in range(DR):
        selswap[DR + i, i] = 1.0                      # extract h1 raw
        selswap[DR + (i + DR // 2) % DR, DR + i] = 1.0  # extract h1 swapped

    # DoubleRow mask operands: chunk0 carries the additive causal mask for
    # the 4 diagonal sub-positions ({0, -448}, exact in fp8e4m3), chunk1 = 0.
    identz = np.zeros((128, 2, 128), dtype=np.float32)
    identz[:, 0, :] = np.eye(128, dtype=np.float32)
    maskdz = np.zeros((128, 4, 2, 512), dtype=np.float32)
    p = np.arange(128)[:, None]
    f = np.arange(512)[None, :]
    for sub in range(4):
        maskdz[:, sub, 0, :] = np.where(p + 128 * sub <= f, 0.0, MASKV)

    # q_b columns per dest: [qn_h0 | qn_h1 | qpe_h0(perm) ; qpe_h1(perm)]
    # NOTE: no SCALE fold — softmax scale is applied inside the exp.
    wqb_all = np.concatenate([
        np.concatenate([
            w_q_b[:, h0 * DQK:h0 * DQK + DN],
            w_q_b[:, h1 * DQK:h1 * DQK + DN],
            w_q_b[:, h0 * DQK + DN:(h0 + 1) * DQK][:, order],
            w_q_b[:, h1 * DQK + DN:(h1 + 1) * DQK][:, order],
        ], axis=1)
        for h0, h1 in ((2 * d, 2 * d + 1) for d in range(NCORES))
    ], axis=1) * q_a_ln_w[:, None]

    def pack(w, mrows):
        Kd, Md = w.shape
        n = Md // mrows
        return np.ascontiguousarray(
            w.reshape(Kd // 128, 128, n, mrows).transpose(2, 1, 0, 3)
            .reshape(n * 128, (Kd // 128) * mrows))

    wqa_pk = pack(w_q_a, 128)
    wkva_pk = pack(wkva_p[:, :KVLR], 128)
    wkpe_pk = pack(wkva_p[:, KVLR:], DR)
    wqb_pk = pack(wqb_all, QCH)

    def bf(x):
        return np.ascontiguousarray(np.asarray(x, dtype=np.float32)).astype(BF_NP)

    def f8(x):
        return np.ascontiguousarray(np.asarray(x, dtype=np.float32)).astype(F8_NP)

    in_maps = []
    for c in range(NCORES):
        h0, h1 = HPC * c, HPC * c + 1
        # own-head kv_b columns: [kn_h0 | kn_h1 | v_h0 | v_h1], ln folded
        wkvb_own = np.concatenate([
            w_kv_b[:, h0 * (DN + DV):h0 * (DN + DV) + DN],
            w_kv_b[:, h1 * (DN + DV):h1 * (DN + DV) + DN],
            w_kv_b[:, h0 * (DN + DV) + DN:(h0 + 1) * (DN + DV)],
            w_kv_b[:, h1 * (DN + DV) + DN:(h1 + 1) * (DN + DV)],
        ], axis=1) * kv_a_ln_w[:, None]
        wkvb_pk = pack(wkvb_own, 4 * 128)
        wo_c = np.concatenate([
            w_o[h0 * DV:(h0 + 1) * DV, :],
            w_o[h1 * DV:(h1 + 1) * DV, :],
        ], axis=0)
        tsl = slice(c * TSH, (c + 1) * TSH)
        in_maps.append({
            "hs_sh": bf(hs_t[:, tsl]),
            "hs_full": bf(hs_t),
            "cosk": bf(cosf),
            "sink": bf(sinf),
            "wqa": bf(wqa_pk),
            "wkva": bf(wkva_pk),
            "wkpe": bf(wkpe_pk),
            "wqb": bf(wqb_pk),
            "wkvb": bf(wkvb_pk),
            "wo": bf(wo_c),
            "cosf2": bf(cosf2[:, tsl]),
            "sinf2": bf(sinf2[:, tsl]),
            "perm128": bf(perm128),
            "selswap": bf(selswap),
            "identz": f8(identz.reshape(128, 2 * 128)),
            "maskdz": f8(maskdz.reshape(128, 4 * 1024)),
            "ones": bf(np.ones((128, 128), dtype=np.float32)),
        })
    return in_maps


def kernel(positions, hidden_states, w_q_a, q_a_ln_w, w_q_b, w_kv_a,
           kv_a_ln_w, w_kv_b, w_o):
    nc = _get_nc()
    in_maps = make_in_maps(positions, hidden_states, w_q_a, q_a_ln_w, w_q_b,
                           w_kv_a, kv_a_ln_w, w_kv_b, w_o)
    res = bass_utils.run_bass_kernel_spmd(nc, in_maps, core_ids=list(range(NCORES)))
    acc = np.zeros((T, HID), dtype=np.float32)
    for c in range(NCORES):
        acc += np.asarray(res.results[c]["out"], dtype=np.float32)
    return acc


# revision 51
# speedup vs baseline: 1.0902x; 1.0025x over previous
"""DeepseekV2 MLA attention on 8 Trainium2 NeuronCores (Bass/Tile), v7.

Token-sharded front end (bf16 q_a/q_b/kv_a for accuracy); the 576-row kv
latent (normalized kv_a + roped k_pe) is AllGathered early — the Pool queue
carries ONLY the collectives so the AllGather fires as soon as the latent
is staged (~13us).  q_b outputs are exchanged per head in fp8 as 256 rows
per dest [qn(128) | qpe(64) | qpe_resid(64)]: the residual rides the
otherwise-wasted pad half of the DoubleRow pe-chunk and cancels the fp8
quantization of q_pe.  Scores run as fp8e4 DoubleRow matmuls
(lhsT=(kn | kpe,kpe-copy), rhs=(qn | qpe,resid), 2x128 contraction per
instruction at 0.5 cyc/row); the causal mask is added in PSUM by a DR
(ident,0)x(maskd,0) matmul with exact fp8 constants {0,-448}; the softmax
SCALE is applied inside the exp activation.  exp runs once per ki-pair on
[128,2,512] PSUM tiles.  v/e/attnV/w_o stay bf16 (fp8 v measurably breaks
the 2e-2 gate); z = ones^T . esum with DVE pair-sums.  Row-parallel w_o;
host sums the 8 bf16 partials in fp32.
"""

import numpy as np
import ml_dtypes

import concourse.bass as bass
import concourse.bacc as bacc
import concourse.mybir as mybir
import concourse.tile as tile
from concourse import bass_utils

T = 2048
HID = 2048
H = 16
DN = 128
DR = 64
DV = 128
DQK = DN + DR
QLR = 1536
KVLR = 512
THETA = 10000.0
EPS = 1e-6
SCALE = DQK ** -0.5

NCORES = 8
HPC = H // NCORES
LATR = KVLR + DR          # 576 rows of exchanged kv latent

F32 = mybir.dt.float32
BF = mybir.dt.bfloat16
F8 = mybir.dt.float8e4
BF_NP = ml_dtypes.bfloat16
F8_NP = ml_dtypes.float8_e4m3
DRMODE = mybir.MatmulPerfMode.DoubleRow

KT = HID // 128           # 16 contraction strips over hidden
QMT = QLR // 128          # 12
KVMT = KVLR // 128        # 4
NB = T // 512             # 4 query blocks
TBT = T // 128            # 16 token blocks
TSH = T // NCORES         # 256 tokens per shard

QCH = 3 * 128             # 384 q_b output rows per dest (qn0,qn1,pe-pair)
QEX = 256                 # exchanged rows per dest per head
MASKV = -240.0            # max-finite of IEEE e4m3; -240*SCALE = -17.3 in exp


def build_bass():
    nc = bacc.Bacc(
        "TRN2",
        target_bir_lowering=False,
        debug=False,
        enable_asserts=False,
        num_devices=NCORES,
    )

    hs_sh = nc.dram_tensor("hs_sh", [HID, TSH], BF, kind="ExternalInput").ap()
    hs_full = nc.dram_tensor("hs_full", [KT * 128, T], BF, kind="ExternalInput").ap()
    wqa = nc.dram_tensor("wqa", [QMT * 128, KT * 128], BF, kind="ExternalInput").ap()
    wkva = nc.dram_tensor("wkva", [KVMT * 128, KT * 128], BF, kind="ExternalInput").ap()
    wkpe = nc.dram_tensor("wkpe", [128, KT * DR], BF, kind="ExternalInput").ap()
    wqb = nc.dram_tensor("wqb", [NCORES * 128, QMT * QCH], BF, kind="ExternalInput").ap()
    wkvb = nc.dram_tensor("wkvb", [128, KVMT * 4 * 128], BF, kind="ExternalInput").ap()
    wo = nc.dram_tensor("wo", [HPC * DV, HID], BF, kind="ExternalInput").ap()
    cosf2 = nc.dram_tensor("cosf2", [128, TSH], BF, kind="ExternalInput").ap()
    sinf2 = nc.dram_tensor("sinf2", [128, TSH], BF, kind="ExternalInput").ap()
    cosk = nc.dram_tensor("cosk", [DR, T], BF, kind="ExternalInput").ap()
    sink = nc.dram_tensor("sink", [DR, T], BF, kind="ExternalInput").ap()
    perm128 = nc.dram_tensor("perm128", [128, 128], BF, kind="ExternalInput").ap()
    selswap = nc.dram_tensor("selswap", [128, 128], BF, kind="ExternalInput").ap()
    identz = nc.dram_tensor("identz", [128, 2 * 128], F8, kind="ExternalInput").ap()
    maskdz = nc.dram_tensor("maskdz", [128, 4 * 1024], F8, kind="ExternalInput").ap()
    ones = nc.dram_tensor("ones", [128, 128], BF, kind="ExternalInput").ap()
    out = nc.dram_tensor("out", [T, HID], BF, kind="ExternalOutput").ap()

    with tile.TileContext(nc) as tc:
        _kernel_body(nc, tc, hs_sh, hs_full, wqa, wkva, wkpe, wqb, wkvb, wo,
                     cosf2, sinf2, cosk, sink, perm128, selswap, identz,
                     maskdz, ones, out)

    nc.compile()
    return nc


def _kernel_body(nc, tc, hs_sh, hs_full, wqa, wkva, wkpe, wqb, wkvb, wo,
                 cosf2, sinf2, cosk, sink, perm128, selswap, identz, maskdz,
                 ones, out):
    from contextlib import ExitStack

    ctx = ExitStack()
    with ctx:
        dram = ctx.enter_context(tc.tile_pool(name="dram", bufs=1, space="DRAM"))
        contrib_kv = dram.tile([KVLR, TSH], BF)
        a2a_kv = dram.tile([NCORES * KVLR, TSH], BF)
        contrib_qh = [dram.tile([NCORES * QEX, TSH], F8, name=f"cq{h}")
                      for h in range(HPC)]
        a2a_qh = [dram.tile([NCORES * QEX, TSH], F8, name=f"aq{h}")
                  for h in range(HPC)]

        persist = ctx.enter_context(tc.tile_pool(name="persist", bufs=1))
        # persist DMAs ride the Act queue behind hs2/wkva1/wkva3 so SP is
        # free for the rest of the AllGather-critical path.
        ones_t = persist.tile([128, 128], BF, tag="ones")
        cos_t = persist.tile([128, TSH], BF, tag="cos")
        sin_t = persist.tile([128, TSH], BF, tag="sin")
        identz_t = persist.tile([128, 2, 128], F8, tag="identz")
        maskdz_t = persist.tile([128, 4, 2, 512], F8, tag="maskdz")
        wkvb_t = persist.tile([128, KVMT, 4 * 128], BF, tag="wkvb")
        perm_t = persist.tile([128, 128], BF, tag="perm")
        selswap_t = persist.tile([128, 128], BF, tag="selswap")
        wo_t = [persist.tile([128, HID], BF, tag=f"wo{h}", name=f"wo{h}")
                for h in range(HPC)]
        wq_t = []
        for d in range(NCORES):
            wq_t.append(persist.tile([128, QMT * QCH], BF, tag=f"wq{d}",
                                     name=f"wq{d}"))
        ones_col = ones_t[:, 0:1]
        ones_row = ones_t[0:1, :]

        cosk_t = persist.tile([DR, T], BF, tag="cosk")
        sink_t = persist.tile([DR, T], BF, tag="sink")

        def _persist_early():
            # needed by the kv-latent critical path (rope, rsqrt broadcast)
            # and the q_b rope (perm/selswap)
            nc.scalar.dma_start(out=ones_t, in_=ones)
            nc.scalar.dma_start(out=cos_t, in_=cosf2)
            nc.scalar.dma_start(out=sin_t, in_=sinf2)
            nc.scalar.dma_start(out=perm_t, in_=perm128)
            nc.scalar.dma_start(out=selswap_t, in_=selswap)
            # preload the Sqrt act-func set off the critical path
            actwarm = persist.tile([1, 8], F32, tag="actwarm")
            nc.scalar.activation(actwarm, ones_t[0:1, 0:8],
                                 mybir.ActivationFunctionType.Sqrt)

        def _persist_late():
            # emitted after the AllGather is issued, on SP behind the wq
            # stream — none of these are needed before ~60us
            nc.sync.dma_start(
                out=identz_t, in_=identz.rearrange("p (c k) -> p c k", c=2))
            nc.sync.dma_start(
                out=maskdz_t,
                in_=maskdz.rearrange("p (s c f) -> p s c f", s=4, c=2))
            nc.sync.dma_start(
                out=wkvb_t, in_=wkvb.rearrange("p (s c) -> p s c", s=KVMT))
            nc.sync.dma_start(out=cosk_t, in_=cosk)
            nc.sync.dma_start(out=sink_t, in_=sink)
            for h in range(HPC):
                nc.sync.dma_start(out=wo_t[h],
                                  in_=wo[h * DV:(h + 1) * DV, :])

        pmid = ctx.enter_context(tc.tile_pool(name="pmid", bufs=1))

        # ---- Phase A: latents on own shard --------------------------------
        # Pool queue carries ONLY the collectives: the AllGather is issued
        # first and fires as soon as its contrib DMAs (on the DVE queue)
        # complete.  Weight DMAs ride SP/Act around the critical path.
        with tc.tile_pool(name="pa", bufs=1) as pa, \
             tc.tile_pool(name="psa", bufs=1, space="PSUM") as psa:
            # warm-up: memset feeds dummy matmuls that ramp the PE p-state
            # while the first DMAs land (cost model: 3us of continuous PE
            # execution reaches full clock).
            warm_t = pa.tile([128, 256], BF, tag="warm")
            nc.vector.memset(warm_t, 1.0)
            warm_ps = psa.tile([128, TSH], F32, tag="pq", bufs=3,
                               name="warm_ps")
            for i in range(44):
                nc.tensor.matmul(warm_ps[0:1, 0:64], lhsT=warm_t[:, 0:1],
                                 rhs=warm_t[:, 0:64], start=True, stop=True,
                                 skip_group_check=True)
            # hs/wkva split across SP and Act so the four kv_a strips land
            # by ~5us; everything else queues behind them.
            wkva4_t = pa.tile([128, KVMT, KT * 128], BF, tag="wkva4")
            hs_t = pa.tile([128, KT, TSH], BF, tag="hst")
            nc.sync.dma_start(out=wkva4_t[:, 0, :], in_=wkva[0:128, :])
            nc.scalar.dma_start(
                out=hs_t[:, 0:KT // 2, :],
                in_=hs_sh[0:HID // 2].rearrange("(k p) t -> p k t", p=128))
            nc.sync.dma_start(
                out=hs_t[:, KT // 2:, :],
                in_=hs_sh[HID // 2:].rearrange("(k p) t -> p k t", p=128))
            nc.scalar.dma_start(out=wkva4_t[:, 1, :], in_=wkva[128:256, :])
            nc.sync.dma_start(out=wkva4_t[:, 2, :], in_=wkva[256:384, :])
            nc.scalar.dma_start(out=wkva4_t[:, 3, :], in_=wkva[384:, :])
            wkpe_t = persist.tile([128, KT * DR], BF, tag="wkpe")
            nc.sync.dma_start(out=wkpe_t, in_=wkpe)
            _persist_early()
            hst = [hs_t[:, k, :] for k in range(KT)]
            wkva_t = [wkva4_t[:, m, :] for m in range(KVMT)]

            def rsqrt_bc(z_psum, n, tag):
                # rsqrt(z/n + eps) = sqrt(n / (z + n*eps)): DVE add+recip,
                # one Act Sqrt hop (Act queue is kept clear of big DMAs here)
                tmp = pa.tile([1, TSH], F32, tag="rsq_tmp", bufs=2)
                nc.vector.tensor_scalar_add(tmp, z_psum, n * EPS)
                nc.vector.reciprocal(tmp, tmp)
                srow = pa.tile([1, TSH], BF, tag=tag + "r", name=tag + "r")
                nc.scalar.activation(srow, tmp,
                                     mybir.ActivationFunctionType.Sqrt,
                                     scale=float(n))
                b_ps = psa.tile([128, TSH], F32, tag="bc", bufs=1,
                                name="b_ps")
                nc.tensor.matmul(b_ps, lhsT=ones_row, rhs=srow,
                                 start=True, stop=True)
                bc = pmid.tile([128, TSH], BF, tag=tag, name=tag)
                nc.vector.tensor_copy(bc, b_ps)
                return bc

            zkv = psa.tile([1, TSH], F32, tag="zkv")
            kv_raw = []   # bf16 un-normalized latent strips
            for m in range(KVMT):
                pq = psa.tile([128, TSH], F32, tag="pq", bufs=3)
                for k in range(KT):
                    nc.tensor.matmul(pq, lhsT=wkva_t[m][:, k * 128:(k + 1) * 128],
                                     rhs=hst[k],
                                     start=(k == 0), stop=(k == KT - 1))
                st = pa.tile([128, TSH], BF, tag=f"kvr{m}", name=f"kvr{m}")
                nc.vector.tensor_copy(st, pq)
                kv_raw.append(st)
                sq = pa.tile([128, TSH], BF, tag="sq", bufs=2)
                nc.vector.tensor_tensor(sq, st, st, op=mybir.AluOpType.mult)
                nc.tensor.matmul(zkv, lhsT=ones_col, rhs=sq,
                                 start=(m == 0), stop=(m == KVMT - 1))

            skv_bc = rsqrt_bc(zkv, KVLR, "skvbc")
            # normalized latent staged contiguously for one contrib DMA
            kvstage = pa.tile([128, KVMT, TSH], BF, tag="kvstage")
            for m in range(KVMT):
                nc.vector.tensor_tensor(kvstage[:, m, :], kv_raw[m], skv_bc,
                                        op=mybir.AluOpType.mult)

            # contrib DMA rides the Pool queue itself — idle, dedicated, and
            # immediately ahead of the AllGather, so no other ready work can
            # steal its slot.  (k_pe is computed replicated, not exchanged.)
            nc.gpsimd.dma_start(
                out=contrib_kv.rearrange("(g p) t -> p g t", p=128),
                in_=kvstage)
            nc.gpsimd.collective_compute(
                "AllGather", mybir.AluOpType.bypass,
                replica_groups=[list(range(NCORES))],
                ins=[contrib_kv], outs=[a2a_kv])
            _persist_late()

            # q_b weights on SP behind the front; q_a strips on Act
            for d in range(NCORES):
                nc.sync.dma_start(out=wq_t[d],
                                  in_=wqb[d * 128:(d + 1) * 128, :])

            # q latent
            zq = psa.tile([1, TSH], F32, tag="zq")
            q_raw = []
            for m in range(QMT):
                wt = pa.tile([128, KT * 128], BF, tag="wqa", bufs=4)
                nc.scalar.dma_start(out=wt, in_=wqa[m * 128:(m + 1) * 128, :])
                pq = psa.tile([128, TSH], F32, tag="pq", bufs=3)
                for k in range(KT):
                    nc.tensor.matmul(pq, lhsT=wt[:, k * 128:(k + 1) * 128],
                                     rhs=hst[k],
                                     start=(k == 0), stop=(k == KT - 1))
                st = pmid.tile([128, TSH], BF, tag=f"qr{m}", name=f"qr{m}")
                nc.vector.tensor_copy(st, pq)
                q_raw.append(st)
                sq = pa.tile([128, TSH], BF, tag="sq", bufs=2)
                nc.vector.tensor_tensor(sq, st, st, op=mybir.AluOpType.mult)
                nc.tensor.matmul(zq, lhsT=ones_col, rhs=sq,
                                 start=(m == 0), stop=(m == QMT - 1))
            sq_bc = rsqrt_bc(zq, QLR, "sqbc")
            # preload the Exp act-func set well before the first real exp
            actwarm2 = pa.tile([1, 8], F32, tag="actwarm2")
            nc.scalar.activation(actwarm2, ones_t[0:1, 0:8],
                                 mybir.ActivationFunctionType.Exp)
            qan = []
            for m in range(QMT):
                qq_ = pmid.tile([128, TSH], BF, tag=f"qan{m}", name=f"qan{m}")
                nc.vector.tensor_tensor(qq_, q_raw[m], sq_bc,
                                        op=mybir.AluOpType.mult)
                qan.append(qq_)

        bcp = ctx.enter_context(tc.tile_pool(name="bcp", bufs=1))
        # kk: DoubleRow score lhsT per head: [dims, shard, half, slot, 128tok]
        # slots per token block: [kn_h0 | kpe-pair | kn_h1 | kpe-pair]
        # head h uses slots (2h, 2h+1); slot 1 == slot 3 = [kpe ; kpe-copy].
        kk = bcp.tile([128, NCORES, 2, 4, 128], F8, tag="kk", name="kk")
        kpe8 = bcp.tile([DR, NCORES, 2, 128], F8, tag="kpe8", name="kpe8")

        # ---- q_b for all dests + per-head exchange ------------------------
        # head-0 AllToAll goes first so head-0 attention can overlap the
        # head-1 AllToAll.  Exchange rows per dest: [qn128 | qpe64 | resid64].
        # The replicated-k_pe work shares these pools so nothing serializes
        # on pool open/close.
        with tc.tile_pool(name="pw", bufs=1) as pw, \
             tc.tile_pool(name="psw", bufs=1, space="PSUM") as psw:
            st_qn = [pw.tile([128, NCORES, TSH], F8, tag=f"stqn{h}",
                             name=f"stqn{h}") for h in range(HPC)]
            st_pe = [pw.tile([128, NCORES, TSH], F8, tag=f"stpe{h}",
                             name=f"stpe{h}") for h in range(HPC)]
            cos64 = cos_t[0:DR, :]
            sin64 = sin_t[0:DR, :]
            for d in range(NCORES):
                wq = wq_t[d]
                accq = []
                for mt in range(3):
                    a = psw.tile([128, TSH], F32, tag="acc", bufs=4,
                                 name=f"accq{mt}")
                    accq.append(a)
                for k in range(QMT):
                    for mt in range(3):
                        nc.tensor.matmul(
                            accq[mt],
                            lhsT=wq[:, k * QCH + mt * 128:k * QCH + (mt + 1) * 128],
                            rhs=qan[k],
                            start=(k == 0), stop=(k == QMT - 1))
                for hh in range(HPC):
                    nc.vector.tensor_copy(st_qn[hh][:, d, :], accq[hh])
                # q_pe rope: the permutation is a +-32-row rotation, done as
                # partition-offset half-products on DVE (no PE matmuls)
                qraw = pw.tile([128, TSH], BF, tag="qraw", bufs=2)
                nc.vector.tensor_copy(qraw, accq[2])
                HD = DR // 2
                for hh in range(HPC):
                    base = qraw[hh * DR:(hh + 1) * DR]
                    r1 = pw.tile([DR, TSH], BF, tag=f"r1_{hh}", bufs=2)
                    nc.vector.tensor_tensor(r1, base, cos64,
                                            op=mybir.AluOpType.mult)
                    r2 = pw.tile([DR, TSH], BF, tag=f"r2_{hh}", bufs=2)
                    nc.vector.tensor_tensor(r2[0:HD, :], base[HD:DR, :],
                                            sin64[0:HD, :],
                                            op=mybir.AluOpType.mult)
                    nc.vector.tensor_tensor(r2[HD:DR, :], base[0:HD, :],
                                            sin64[HD:DR, :],
                                            op=mybir.AluOpType.mult)
                    pe_bf = pw.tile([DR, TSH], BF, tag=f"pebf{hh}", bufs=2)
                    nc.vector.tensor_tensor(pe_bf, r1, r2,
                                            op=mybir.AluOpType.add)
                    nc.vector.tensor_copy(st_pe[hh][0:DR, d, :], pe_bf)
                    # residual of the fp8 cast (mixed-dtype subtract)
                    nc.vector.tensor_tensor(st_pe[hh][DR:128, d, :],
                                            pe_bf, st_pe[hh][0:DR, d, :],
                                            op=mybir.AluOpType.subtract)
            for h in range(HPC):
                nc.sync.dma_start(
                    out=contrib_qh[h].rearrange("(d c) t -> c d t",
                                                d=NCORES)[0:128],
                    in_=st_qn[h])
                nc.sync.dma_start(
                    out=contrib_qh[h].rearrange("(d c) t -> c d t",
                                                d=NCORES)[128:QEX],
                    in_=st_pe[h])
                nc.gpsimd.collective_compute(
                    "AllToAll", mybir.AluOpType.bypass,
                    replica_groups=[list(range(NCORES))],
                    ins=[contrib_qh[h]], outs=[a2a_qh[h]])

            # ---- replicated k_pe over all tokens --------------------------
            # Real PE work that fills the AllGather window (and keeps the
            # p-state warm): k_pe = w_kpe^T . hs for all 2048 tokens, roped.
            kpel_all = pw.tile([DR, NCORES, 2, 128], BF, tag="kpelall")
            for qt_ in range(4):
                tslh = slice(qt_ * 512, (qt_ + 1) * 512)
                kpe_ps = psw.tile([DR, 512], F32, tag="kpeps", bufs=2)
                for k in range(KT):
                    hstrip = pw.tile([128, 512], BF, tag="hsf", bufs=14)
                    eng = nc.sync if k % 2 == 0 else nc.scalar
                    eng.dma_start(out=hstrip,
                                  in_=hs_full[k * 128:(k + 1) * 128, tslh])
                    nc.tensor.matmul(kpe_ps,
                                     lhsT=wkpe_t[:, k * DR:(k + 1) * DR],
                                     rhs=hstrip,
                                     start=(k == 0), stop=(k == KT - 1))
                kpe_rawh = pw.tile([DR, 512], BF, tag="kpraw", bufs=2)
                nc.vector.tensor_copy(kpe_rawh, kpe_ps)
                HD = DR // 2
                rt1 = pw.tile([DR, 512], BF, tag="rt1k", bufs=2)
                nc.vector.tensor_tensor(rt1, kpe_rawh, cosk_t[:, tslh],
                                        op=mybir.AluOpType.mult)
                rt2 = pw.tile([DR, 512], BF, tag="rt2k", bufs=2)
                nc.vector.tensor_tensor(rt2[0:HD, :], kpe_rawh[HD:DR, :],
                                        sink_t[0:HD, tslh],
                                        op=mybir.AluOpType.mult)
                nc.vector.tensor_tensor(rt2[HD:DR, :], kpe_rawh[0:HD, :],
                                        sink_t[HD:DR, tslh],
                                        op=mybir.AluOpType.mult)
                nc.vector.tensor_tensor(
                    kpel_all[:, 2 * qt_:2 * qt_ + 2, :, :], rt1, rt2,
                    op=mybir.AluOpType.add)
            nc.vector.tensor_copy(kpe8, kpel_all)
            for pr in range(2):
                dst = kk[pr * DR:(pr + 1) * DR]
                eng = nc.sync if pr == 0 else nc.scalar
                eng.dma_start(out=dst[:, :, :, 1, :], in_=kpe8)
                eng.dma_start(out=dst[:, :, :, 3, :], in_=kpe8)
            # keep the PE p-state warm until the AllGather lands
            wwps = psw.tile([DR, 512], F32, tag="kpeps", bufs=2, name="wwps")
            for i in range(100):
                nc.tensor.matmul(wwps[0:1, 0:256], lhsT=kpe8[:, 0, 0, 0:1],
                                 rhs=kpe8[:, 0, :, :], start=True, stop=True,
                                 skip_group_check=True)


        # ---- Phase B: expand k_nope / v for own heads over all tokens -----
        kvan = []      # latent strips, all tokens [128, 8, 256]
        for r in range(KVMT):
            kt_ = bcp.tile([128, NCORES, TSH], BF, tag=f"kvan{r}",
                           name=f"kvan{r}")
            eng = nc.sync if r % 2 == 0 else nc.scalar
            eng.dma_start(
                out=kt_,
                in_=a2a_kv.rearrange("(s r) t -> r s t", s=NCORES)
                            [r * 128:(r + 1) * 128])
            kvan.append(kt_)

        def tok512(tile3, c):
            # 512-token chunk c of a [*, 8, 256] tile
            return tile3[:, 2 * c:2 * c + 2, :]

        def tok128(tile3, tb):
            half = (tb % 2) * 128
            return tile3[:, tb // 2, half:half + 128]

        vt = [None] * TBT   # per 128-token block [128, HPC*DV] token-major v
        with tc.tile_pool(name="pb", bufs=1) as pb, \
             tc.tile_pool(name="psb", bufs=1, space="PSUM") as psb:
            for h in range(HPC):
                for c in range(4):
                    acck = psb.tile([128, 512], F32, tag="acck", bufs=2)
                    for s in range(KVMT):
                        nc.tensor.matmul(
                            acck, lhsT=wkvb_t[:, s, h * DN:(h + 1) * DN],
                            rhs=tok512(kvan[s], c),
                            start=(s == 0), stop=(s == KVMT - 1))
                    nc.vector.tensor_copy(
                        kk[:, 2 * c:2 * c + 2, :, 2 * h, :]
                        .rearrange("p s j c -> p (s j) c"),
                        acck.rearrange("p (f c) -> p f c", c=128))
            for tb in range(TBT):
                accv = psb.tile([128, HPC * DV], F32, tag="accv", bufs=3)
                for s in range(KVMT):
                    nc.tensor.matmul(
                        accv, lhsT=tok128(kvan[s], tb),
                        rhs=wkvb_t[:, s, 2 * DN:2 * DN + HPC * DV],
                        start=(s == 0), stop=(s == KVMT - 1))
                vt[tb] = bcp.tile([128, HPC * DV], BF, tag=f"v{tb}",
                                  name=f"v{tb}")
                nc.vector.tensor_copy(vt[tb], accv)

        # q tiles for own heads, all tokens: [dims, chunk, shard, 256]
        # chunk0 = qn (128 dims); chunk1 = [qpe 64 | qpe_resid 64].
        qq = []
        for h in range(HPC):
            qt = bcp.tile([128, 2, NCORES, TSH], F8, tag=f"qq{h}",
                          name=f"qq{h}")
            src = a2a_qh[h].rearrange("(s c) t -> c s t", s=NCORES)
            engs = ((nc.sync, nc.scalar) if h == 0 else (nc.gpsimd, nc.sync))
            for sh in range(2):
                ssl = slice(sh * 4, sh * 4 + 4)
                qeng = engs[sh]
                qeng.dma_start(out=qt[:, 0, ssl, :], in_=src[0:128, ssl])
                qeng.dma_start(out=qt[0:DR, 1, ssl, :],
                               in_=src[128:128 + DR, ssl])
                qeng.dma_start(out=qt[DR:128, 1, ssl, :],
                               in_=src[128 + DR:QEX, ssl])
            qq.append(qt)

        # ---- Attention + output projection --------------------------------
        with tc.tile_pool(name="pc", bufs=1) as pc, \
             tc.tile_pool(name="psc", bufs=1, space="PSUM") as psc:
            attn_n = [[None] * NB for _ in range(HPC)]
            for h in range(HPC):
                for qj in reversed(range(NB)):
                    npair = 2 * qj + 2
                    q_rhs = qq[h][:, :, 2 * qj:2 * qj + 2, :]
                    attn_ps = psc.tile([128, 512], F32, tag="attn", bufs=2)
                    esums = []   # tree-reduced pair sums (short dep chains)
                    for p in range(npair):
                        s_pair = psc.tile([128, 2, 512], F32, tag="s", bufs=2)
                        for i in range(2):
                            ki = 2 * p + i
                            diag = ki >= 4 * qj
                            nc.tensor.matmul(
                                s_pair[:, i, :],
                                lhsT=kk[:, ki // 2, ki % 2,
                                        2 * h:2 * h + 2, :],
                                rhs=q_rhs,
                                start=True, stop=not diag,
                                perf_mode=DRMODE)
                            if diag:
                                sub = ki - 4 * qj
                                nc.tensor.matmul(
                                    s_pair[:, i, :],
                                    lhsT=identz_t,
                                    rhs=maskdz_t[:, sub],
                                    start=False, stop=True,
                                    perf_mode=DRMODE)
                        e_pair = pc.tile([128, 2, 512], BF, tag="e", bufs=6)
                        nc.scalar.activation(e_pair, s_pair,
                                             mybir.ActivationFunctionType.Exp,
                                             scale=SCALE)
                        for i in range(2):
                            ki = 2 * p + i
                            nc.tensor.matmul(
                                attn_ps,
                                lhsT=vt[ki][:, h * DV:(h + 1) * DV],
                                rhs=e_pair[:, i, :],
                                start=(p == 0 and i == 0),
                                stop=(p == npair - 1 and i == 1))
                        et = pc.tile([128, 512], BF, tag="etree", bufs=8)
                        nc.vector.tensor_tensor(
                            et, e_pair[:, 0, :], e_pair[:, 1, :],
                            op=mybir.AluOpType.add)
                        esums.append(et)
                        # opportunistic tree combine of completed pairs
                        while len(esums) >= 2 and p < npair - 1:
                            b = esums.pop()
                            a = esums.pop()
                            et2 = pc.tile([128, 512], BF, tag="etree",
                                          bufs=8)
                            nc.vector.tensor_tensor(
                                et2, a, b, op=mybir.AluOpType.add)
                            esums.append(et2)
                            break
                    while len(esums) > 1:
                        b = esums.pop()
                        a = esums.pop()
                        et2 = pc.tile([128, 512], BF, tag="etree", bufs=8)
                        nc.vector.tensor_tensor(et2, a, b,
                                                op=mybir.AluOpType.add)
                        esums.append(et2)
                    esum = esums[0]
                    # z = colsum(esum); broadcast 1/z via ones matmul
                    zrow_ps = psc.tile([128, 512], F32, tag="o", bufs=2,
                                       name="zrow_ps")
                    nc.tensor.matmul(zrow_ps[0:1, :], lhsT=ones_col,
                                     rhs=esum, start=True, stop=True)
                    zr = pc.tile([1, 512], BF, tag="zr", bufs=2)
                    with nc.allow_low_precision(reason="bf16 softmax"):
                        nc.vector.tensor_copy(zr, zrow_ps[0:1, :])
                    zb_ps = psc.tile([128, 512], F32, tag="o", bufs=2,
                                     name="zb_ps")
                    nc.tensor.matmul(zb_ps, lhsT=ones_row, rhs=zr,
                                     start=True, stop=True)
                    rzb = pc.tile([128, 512], BF, tag="rzb", bufs=2)
                    with nc.allow_low_precision(reason="bf16 softmax"):
                        nc.vector.reciprocal(rzb, zb_ps)
                    attn_n[h][qj] = bcp.tile([128, 512], BF,
                                             tag=f"attn{h}_{qj}",
                                             name=f"attn{h}_{qj}")
                    nc.vector.tensor_tensor(attn_n[h][qj], attn_ps, rzb,
                                            op=mybir.AluOpType.mult)

                    if h == HPC - 1:
                        # both heads' attn_n for this query block are ready
                        for tt in range(4):
                            tb = qj * 4 + tt
                            tsl = slice(tt * 128, (tt + 1) * 128)
                            last = (qj == 0 and tt == 3)  # last in exec order
                            o_row = pc.tile([128, HID], BF, tag="orow",
                                            bufs=2)
                            for hb in range(NB):
                                o_ps = psc.tile([128, 512], F32, tag="o",
                                                bufs=2)
                                for hh in range(HPC):
                                    nc.tensor.matmul(
                                        o_ps,
                                        lhsT=attn_n[hh][qj][:, tsl],
                                        rhs=wo_t[hh][:, hb * 512:(hb + 1) * 512],
                                        start=(hh == 0),
                                        stop=(hh == HPC - 1),
                                    )
                                osl = o_row[:, hb * 512:(hb + 1) * 512]
                                if hb % 2 == 0:
                                    nc.vector.tensor_copy(osl, o_ps)
                                else:
                                    nc.gpsimd.tensor_copy(osl, o_ps)
                                if last:
                                    weng = nc.sync if hb % 2 == 0 else nc.scalar
                                    weng.dma_start(
                                        out=out[tb * 128:(tb + 1) * 128,
                                                hb * 512:(hb + 1) * 512],
                                        in_=osl)
                            if not last:
                                nc.sync.dma_start(
                                    out=out[tb * 128:(tb + 1) * 128, :],
                                    in_=o_row)


_NC_CACHE = {}


def _get_nc():
    if "nc" not in _NC_CACHE:
        _NC_CACHE["nc"] = build_bass()
    return _NC_CACHE["nc"]


def make_in_maps(positions, hidden_states, w_q_a, q_a_ln_w, w_q_b, w_kv_a,
                 kv_a_ln_w, w_kv_b, w_o):
    positions = np.asarray(positions)
    hidden_states = np.asarray(hidden_states, dtype=np.float32)
    w_q_a = np.asarray(w_q_a, dtype=np.float32)
    q_a_ln_w = np.asarray(q_a_ln_w, dtype=np.float32)
    w_q_b = np.asarray(w_q_b, dtype=np.float32)
    w_kv_a = np.asarray(w_kv_a, dtype=np.float32)
    kv_a_ln_w = np.asarray(kv_a_ln_w, dtype=np.float32)
    w_kv_b = np.asarray(w_kv_b, dtype=np.float32)
    w_o = np.asarray(w_o, dtype=np.float32)

    hs_t = np.ascontiguousarray(hidden_states.T)

    order = np.concatenate([np.arange(0, DR, 2), np.arange(1, DR, 2)])

    wkva_p = w_kv_a.copy()
    wkva_p[:, KVLR:] = w_kv_a[:, KVLR:][:, order]

    inv_freq = 1.0 / (THETA ** (np.arange(0, DR, 2, dtype=np.float64) / DR))
    ang = positions.astype(np.float64)[:, None] * inv_freq[None, :]
    cosT = np.cos(ang).T.astype(np.float32)
    sinT = np.sin(ang).T.astype(np.float32)
    cosf = np.concatenate([cosT, cosT], axis=0)          # [64, T]
    sinf = np.concatenate([-sinT, sinT], axis=0)
    cosf2 = np.concatenate([cosf, cosf], axis=0)         # [128, T] two heads
    sinf2 = np.concatenate([sinf, sinf], axis=0)

    perm = np.zeros((DR, DR), dtype=np.float32)
    for i in range(DR):
        perm[i, (i + DR // 2) % DR] = 1.0
    perm128 = np.zeros((128, 128), dtype=np.float32)
    perm128[:DR, :DR] = perm
    perm128[DR:, DR:] = perm
    selswap = np.zeros((128, 128), dtype=np.float32)
    for i in range(DR):
        selswap[DR + i, i] = 1.0                      # extract h1 raw
        selswap[DR + (i + DR // 2) % DR, DR + i] = 1.0  # extract h1 swapped

    # DoubleRow mask operands: chunk0 carries the additive causal mask for
    # the 4 diagonal sub-positions ({0, -448}, exact in fp8e4m3), chunk1 = 0.
    identz = np.zeros((128, 2, 128), dtype=np.float32)
    identz[:, 0, :] = np.eye(128, dtype=np.float32)
    maskdz = np.zeros((128, 4, 2, 512), dtype=np.float32)
    p = np.arange(128)[:, None]
    f = np.arange(512)[None, :]
    for sub in range(4):
        maskdz[:, sub, 0, :] = np.where(p + 128 * sub <= f, 0.0, MASKV)

    # q_b columns per dest: [qn_h0 | qn_h1 | qpe_h0(perm) ; qpe_h1(perm)]
    # NOTE: no SCALE fold — softmax scale is applied inside the exp.
    wqb_all = np.concatenate([
        np.concatenate([
            w_q_b[:, h0 * DQK:h0 * DQK + DN],
            w_q_b[:, h1 * DQK:h1 * DQK + DN],
            w_q_b[:, h0 * DQK + DN:(h0 + 1) * DQK][:, order],
            w_q_b[:, h1 * DQK + DN:(h1 + 1) * DQK][:, order],
        ], axis=1)
        for h0, h1 in ((2 * d, 2 * d + 1) for d in range(NCORES))
    ], axis=1) * q_a_ln_w[:, None]

    def pack(w, mrows):
        Kd, Md = w.shape
        n = Md // mrows
        return np.ascontiguousarray(
            w.reshape(Kd // 128, 128, n, mrows).transpose(2, 1, 0, 3)
            .reshape(n * 128, (Kd // 128) * mrows))

    wqa_pk = pack(w_q_a, 128)
    wkva_pk = pack(wkva_p[:, :KVLR], 128)
    wkpe_pk = pack(wkva_p[:, KVLR:], DR)
    wqb_pk = pack(wqb_all, QCH)

    def bf(x):
        return np.ascontiguousarray(np.asarray(x, dtype=np.float32)).astype(BF_NP)

    def f8(x):
        return np.ascontiguousarray(np.asarray(x, dtype=np.float32)).astype(F8_NP)

    in_maps = []
    for c in range(NCORES):
        h0, h1 = HPC * c, HPC * c + 1
        # own-head kv_b columns: [kn_h0 | kn_h1 | v_h0 | v_h1], ln folded
        wkvb_own = np.concatenate([
            w_kv_b[:, h0 * (DN + DV):h0 * (DN + DV) + DN],
            w_kv_b[:, h1 * (DN + DV):h1 * (DN + DV) + DN],
            w_kv_b[:, h0 * (DN + DV) + DN:(h0 + 1) * (DN + DV)],
            w_kv_b[:, h1 * (DN + DV) + DN:(h1 + 1) * (DN + DV)],
        ], axis=1) * kv_a_ln_w[:, None]
        wkvb_pk = pack(wkvb_own, 4 * 128)
        wo_c = np.concatenate([
            w_o[h0 * DV:(h0 + 1) * DV, :],
            w_o[h1 * DV:(h1 + 1) * DV, :],
        ], axis=0)
        tsl = slice(c * TSH, (c + 1) * TSH)
        in_maps.append({
            "hs_sh": bf(hs_t[:, tsl]),
            "hs_full": bf(hs_t),
            "cosk": bf(cosf),
            "sink": bf(sinf),
            "wqa": bf(wqa_pk),
            "wkva": bf(wkva_pk),
            "wkpe": bf(wkpe_pk),
            "wqb": bf(wqb_pk),
            "wkvb": bf(wkvb_pk),
            "wo": bf(wo_c),
            "cosf2": bf(cosf2[:, tsl]),
            "sinf2": bf(sinf2[:, tsl]),
            "perm128": bf(perm128),
            "selswap": bf(selswap),
            "identz": f8(identz.reshape(128, 2 * 128)),
            "maskdz": f8(maskdz.reshape(128, 4 * 1024)),
            "ones": bf(np.ones((128, 128), dtype=np.float32)),
        })
    return in_maps


def kernel(positions, hidden_states, w_q_a, q_a_ln_w, w_q_b, w_kv_a,
           kv_a_ln_w, w_kv_b, w_o):
    nc = _get_nc()
    in_maps = make_in_maps(positions, hidden_states, w_q_a, q_a_ln_w, w_q_b,
                           w_kv_a, kv_a_ln_w, w_kv_b, w_o)
    res = bass_utils.run_bass_kernel_spmd(nc, in_maps, core_ids=list(range(NCORES)))
    acc = np.zeros((T, HID), dtype=np.float32)
    for c in range(NCORES):
        acc += np.asarray(res.results[c]["out"], dtype=np.float32)
    return acc


# revision 52
# speedup vs baseline: 1.1458x; 1.0510x over previous
"""DeepseekV2 MLA attention on 8 Trainium2 NeuronCores (Bass/Tile), v7.

Token-sharded front end (bf16 q_a/q_b/kv_a for accuracy); the 576-row kv
latent (normalized kv_a + roped k_pe) is AllGathered early — the Pool queue
carries ONLY the collectives so the AllGather fires as soon as the latent
is staged (~13us).  q_b outputs are exchanged per head in fp8 as 256 rows
per dest [qn(128) | qpe(64) | qpe_resid(64)]: the residual rides the
otherwise-wasted pad half of the DoubleRow pe-chunk and cancels the fp8
quantization of q_pe.  Scores run as fp8e4 DoubleRow matmuls
(lhsT=(kn | kpe,kpe-copy), rhs=(qn | qpe,resid), 2x128 contraction per
instruction at 0.5 cyc/row); the causal mask is added in PSUM by a DR
(ident,0)x(maskd,0) matmul with exact fp8 constants {0,-448}; the softmax
SCALE is applied inside the exp activation.  exp runs once per ki-pair on
[128,2,512] PSUM tiles.  v/e/attnV/w_o stay bf16 (fp8 v measurably breaks
the 2e-2 gate); z = ones^T . esum with DVE pair-sums.  Row-parallel w_o;
host sums the 8 bf16 partials in fp32.
"""

import numpy as np
import ml_dtypes

import concourse.bass as bass
import concourse.bacc as bacc
import concourse.mybir as mybir
import concourse.tile as tile
from concourse import bass_utils

T = 2048
HID = 2048
H = 16
DN = 128
DR = 64
DV = 128
DQK = DN + DR
QLR = 1536
KVLR = 512
THETA = 10000.0
EPS = 1e-6
SCALE = DQK ** -0.5

NCORES = 8
HPC = H // NCORES
LATR = KVLR + DR          # 576 rows of exchanged kv latent

F32 = mybir.dt.float32
BF = mybir.dt.bfloat16
F8 = mybir.dt.float8e4
BF_NP = ml_dtypes.bfloat16
F8_NP = ml_dtypes.float8_e4m3
DRMODE = mybir.MatmulPerfMode.DoubleRow

KT = HID // 128           # 16 contraction strips over hidden
QMT = QLR // 128          # 12
KVMT = KVLR // 128        # 4
NB = T // 512             # 4 query blocks
TBT = T // 128            # 16 token blocks
TSH = T // NCORES         # 256 tokens per shard

QCH = 3 * 128             # 384 q_b output rows per dest (qn0,qn1,pe-pair)
QEX = 256                 # exchanged rows per dest per head
MASKV = -240.0            # max-finite of IEEE e4m3; -240*SCALE = -17.3 in exp


def build_bass():
    nc = bacc.Bacc(
        "TRN2",
        target_bir_lowering=False,
        debug=False,
        enable_asserts=False,
        num_devices=NCORES,
    )

    hs_sh = nc.dram_tensor("hs_sh", [HID, TSH], BF, kind="ExternalInput").ap()
    hs_full = nc.dram_tensor("hs_full", [KT * 128, T], BF, kind="ExternalInput").ap()
    wqa = nc.dram_tensor("wqa", [QMT * 128, KT * 128], BF, kind="ExternalInput").ap()
    wkva = nc.dram_tensor("wkva", [KVMT * 128, KT * 128], BF, kind="ExternalInput").ap()
    wkpe = nc.dram_tensor("wkpe", [128, KT * DR], BF, kind="ExternalInput").ap()
    wqb = nc.dram_tensor("wqb", [NCORES * 128, QMT * QCH], BF, kind="ExternalInput").ap()
    wkvb = nc.dram_tensor("wkvb", [128, KVMT * 4 * 128], BF, kind="ExternalInput").ap()
    wo = nc.dram_tensor("wo", [HPC * DV, HID], BF, kind="ExternalInput").ap()
    cosf2 = nc.dram_tensor("cosf2", [128, TSH], BF, kind="ExternalInput").ap()
    sinf2 = nc.dram_tensor("sinf2", [128, TSH], BF, kind="ExternalInput").ap()
    cosk = nc.dram_tensor("cosk", [DR, T], BF, kind="ExternalInput").ap()
    sink = nc.dram_tensor("sink", [DR, T], BF, kind="ExternalInput").ap()
    perm128 = nc.dram_tensor("perm128", [128, 128], BF, kind="ExternalInput").ap()
    selswap = nc.dram_tensor("selswap", [128, 128], BF, kind="ExternalInput").ap()
    identz = nc.dram_tensor("identz", [128, 2 * 128], F8, kind="ExternalInput").ap()
    maskdz = nc.dram_tensor("maskdz", [128, 4 * 1024], F8, kind="ExternalInput").ap()
    ones = nc.dram_tensor("ones", [128, 128], BF, kind="ExternalInput").ap()
    out = nc.dram_tensor("out", [T, HID], BF, kind="ExternalOutput").ap()

    with tile.TileContext(nc) as tc:
        _kernel_body(nc, tc, hs_sh, hs_full, wqa, wkva, wkpe, wqb, wkvb, wo,
                     cosf2, sinf2, cosk, sink, perm128, selswap, identz,
                     maskdz, ones, out)

    nc.compile()
    return nc


def _kernel_body(nc, tc, hs_sh, hs_full, wqa, wkva, wkpe, wqb, wkvb, wo,
                 cosf2, sinf2, cosk, sink, perm128, selswap, identz, maskdz,
                 ones, out):
    from contextlib import ExitStack

    ctx = ExitStack()
    with ctx:
        dram = ctx.enter_context(tc.tile_pool(name="dram", bufs=1, space="DRAM"))
        contrib_kv = dram.tile([KVLR, TSH], BF)
        a2a_kv = dram.tile([NCORES * KVLR, TSH], BF)
        contrib_qh = [dram.tile([NCORES * QEX, TSH], F8, name=f"cq{h}")
                      for h in range(HPC)]
        a2a_qh = [dram.tile([NCORES * QEX, TSH], F8, name=f"aq{h}")
                  for h in range(HPC)]

        persist = ctx.enter_context(tc.tile_pool(name="persist", bufs=1))
        # persist DMAs ride the Act queue behind hs2/wkva1/wkva3 so SP is
        # free for the rest of the AllGather-critical path.
        ones_t = persist.tile([128, 128], BF, tag="ones")
        cos_t = persist.tile([128, TSH], BF, tag="cos")
        sin_t = persist.tile([128, TSH], BF, tag="sin")
        identz_t = persist.tile([128, 2, 128], F8, tag="identz")
        maskdz_t = persist.tile([128, 4, 2, 512], F8, tag="maskdz")
        wkvb_t = persist.tile([128, KVMT, 4 * 128], BF, tag="wkvb")
        perm_t = persist.tile([128, 128], BF, tag="perm")
        selswap_t = persist.tile([128, 128], BF, tag="selswap")
        wo_t = [persist.tile([128, HID], BF, tag=f"wo{h}", name=f"wo{h}")
                for h in range(HPC)]
        wq_t = []
        for d in range(NCORES):
            wq_t.append(persist.tile([128, QMT * QCH], BF, tag=f"wq{d}",
                                     name=f"wq{d}"))
        ones_col = ones_t[:, 0:1]
        ones_row = ones_t[0:1, :]

        cosk_t = persist.tile([DR, T], BF, tag="cosk")
        sink_t = persist.tile([DR, T], BF, tag="sink")

        def _persist_early():
            # needed by the kv-latent critical path (rope, rsqrt broadcast)
            # and the q_b rope (perm/selswap)
            nc.scalar.dma_start(out=ones_t, in_=ones)
            nc.scalar.dma_start(out=cos_t, in_=cosf2)
            nc.scalar.dma_start(out=sin_t, in_=sinf2)
            nc.scalar.dma_start(out=perm_t, in_=perm128)
            nc.scalar.dma_start(out=selswap_t, in_=selswap)
            # preload the Sqrt act-func set off the critical path
            actwarm = persist.tile([1, 8], F32, tag="actwarm")
            nc.scalar.activation(actwarm, ones_t[0:1, 0:8],
                                 mybir.ActivationFunctionType.Sqrt)

        def _persist_late():
            # emitted after the AllGather is issued, on SP behind the wq
            # stream — none of these are needed before ~60us
            nc.sync.dma_start(
                out=identz_t, in_=identz.rearrange("p (c k) -> p c k", c=2))
            nc.sync.dma_start(
                out=maskdz_t,
                in_=maskdz.rearrange("p (s c f) -> p s c f", s=4, c=2))
            nc.sync.dma_start(
                out=wkvb_t, in_=wkvb.rearrange("p (s c) -> p s c", s=KVMT))
            nc.sync.dma_start(out=cosk_t, in_=cosk)
            nc.sync.dma_start(out=sink_t, in_=sink)
            for h in range(HPC):
                nc.sync.dma_start(out=wo_t[h],
                                  in_=wo[h * DV:(h + 1) * DV, :])

        pmid = ctx.enter_context(tc.tile_pool(name="pmid", bufs=1))

        # ---- Phase A: latents on own shard --------------------------------
        # Pool queue carries ONLY the collectives: the AllGather is issued
        # first and fires as soon as its contrib DMAs (on the DVE queue)
        # complete.  Weight DMAs ride SP/Act around the critical path.
        with tc.tile_pool(name="pa", bufs=1) as pa, \
             tc.tile_pool(name="psa", bufs=1, space="PSUM") as psa:
            # warm-up: memset feeds dummy matmuls that ramp the PE p-state
            # while the first DMAs land (cost model: 3us of continuous PE
            # execution reaches full clock).
            warm_t = pa.tile([128, 256], BF, tag="warm")
            nc.vector.memset(warm_t, 1.0)
            warm_ps = psa.tile([128, TSH], F32, tag="pq", bufs=3,
                               name="warm_ps")
            for i in range(44):
                nc.tensor.matmul(warm_ps[0:1, 0:64], lhsT=warm_t[:, 0:1],
                                 rhs=warm_t[:, 0:64], start=True, stop=True,
                                 skip_group_check=True)
            # hs/wkva split across SP and Act so the four kv_a strips land
            # by ~5us; everything else queues behind them.
            wkva4_t = pa.tile([128, KVMT, KT * 128], BF, tag="wkva4")
            hs_t = pa.tile([128, KT, TSH], BF, tag="hst")
            nc.sync.dma_start(out=wkva4_t[:, 0, :], in_=wkva[0:128, :])
            nc.scalar.dma_start(
                out=hs_t[:, 0:KT // 2, :],
                in_=hs_sh[0:HID // 2].rearrange("(k p) t -> p k t", p=128))
            nc.sync.dma_start(
                out=hs_t[:, KT // 2:, :],
                in_=hs_sh[HID // 2:].rearrange("(k p) t -> p k t", p=128))
            nc.scalar.dma_start(out=wkva4_t[:, 1, :], in_=wkva[128:256, :])
            nc.sync.dma_start(out=wkva4_t[:, 2, :], in_=wkva[256:384, :])
            nc.scalar.dma_start(out=wkva4_t[:, 3, :], in_=wkva[384:, :])
            wkpe_t = persist.tile([128, KT * DR], BF, tag="wkpe")
            nc.sync.dma_start(out=wkpe_t, in_=wkpe)
            _persist_early()
            hst = [hs_t[:, k, :] for k in range(KT)]
            wkva_t = [wkva4_t[:, m, :] for m in range(KVMT)]

            def rsqrt_bc(z_psum, n, tag):
                # rsqrt(z/n + eps) = sqrt(n / (z + n*eps)): DVE add+recip,
                # one Act Sqrt hop (Act queue is kept clear of big DMAs here)
                tmp = pa.tile([1, TSH], F32, tag="rsq_tmp", bufs=2)
                nc.vector.tensor_scalar_add(tmp, z_psum, n * EPS)
                nc.vector.reciprocal(tmp, tmp)
                srow = pa.tile([1, TSH], BF, tag=tag + "r", name=tag + "r")
                nc.scalar.activation(srow, tmp,
                                     mybir.ActivationFunctionType.Sqrt,
                                     scale=float(n))
                b_ps = psa.tile([128, TSH], F32, tag="bc", bufs=1,
                                name="b_ps")
                nc.tensor.matmul(b_ps, lhsT=ones_row, rhs=srow,
                                 start=True, stop=True)
                bc = pmid.tile([128, TSH], BF, tag=tag, name=tag)
                nc.vector.tensor_copy(bc, b_ps)
                return bc

            zkv = psa.tile([1, TSH], F32, tag="zkv")
            kv_raw = []   # bf16 un-normalized latent strips
            for m in range(KVMT):
                pq = psa.tile([128, TSH], F32, tag="pq", bufs=3)
                for k in range(KT):
                    nc.tensor.matmul(pq, lhsT=wkva_t[m][:, k * 128:(k + 1) * 128],
                                     rhs=hst[k],
                                     start=(k == 0), stop=(k == KT - 1))
                st = pa.tile([128, TSH], BF, tag=f"kvr{m}", name=f"kvr{m}")
                nc.vector.tensor_copy(st, pq)
                kv_raw.append(st)
                sq = pa.tile([128, TSH], BF, tag="sq", bufs=2)
                nc.vector.tensor_tensor(sq, st, st, op=mybir.AluOpType.mult)
                nc.tensor.matmul(zkv, lhsT=ones_col, rhs=sq,
                                 start=(m == 0), stop=(m == KVMT - 1))

            skv_bc = rsqrt_bc(zkv, KVLR, "skvbc")
            # normalized latent staged contiguously for one contrib DMA
            kvstage = pa.tile([128, KVMT, TSH], BF, tag="kvstage")
            for m in range(KVMT):
                nc.vector.tensor_tensor(kvstage[:, m, :], kv_raw[m], skv_bc,
                                        op=mybir.AluOpType.mult)

            # contrib DMA rides the Pool queue itself — idle, dedicated, and
            # immediately ahead of the AllGather, so no other ready work can
            # steal its slot.  (k_pe is computed replicated, not exchanged.)
            nc.gpsimd.dma_start(
                out=contrib_kv.rearrange("(g p) t -> p g t", p=128),
                in_=kvstage)
            nc.gpsimd.collective_compute(
                "AllGather", mybir.AluOpType.bypass,
                replica_groups=[list(range(NCORES))],
                ins=[contrib_kv], outs=[a2a_kv])

            # q_b weights on SP behind the front; q_a strips on Act
            for d in range(NCORES):
                nc.sync.dma_start(out=wq_t[d],
                                  in_=wqb[d * 128:(d + 1) * 128, :])
            _persist_late()

            # q latent
            zq = psa.tile([1, TSH], F32, tag="zq")
            q_raw = []
            for m in range(QMT):
                wt = pa.tile([128, KT * 128], BF, tag="wqa", bufs=4)
                nc.scalar.dma_start(out=wt, in_=wqa[m * 128:(m + 1) * 128, :])
                pq = psa.tile([128, TSH], F32, tag="pq", bufs=3)
                for k in range(KT):
                    nc.tensor.matmul(pq, lhsT=wt[:, k * 128:(k + 1) * 128],
                                     rhs=hst[k],
                                     start=(k == 0), stop=(k == KT - 1))
                st = pmid.tile([128, TSH], BF, tag=f"qr{m}", name=f"qr{m}")
                nc.vector.tensor_copy(st, pq)
                q_raw.append(st)
                sq = pa.tile([128, TSH], BF, tag="sq", bufs=2)
                nc.vector.tensor_tensor(sq, st, st, op=mybir.AluOpType.mult)
                nc.tensor.matmul(zq, lhsT=ones_col, rhs=sq,
                                 start=(m == 0), stop=(m == QMT - 1))
            sq_bc = rsqrt_bc(zq, QLR, "sqbc")
            # preload the Exp act-func set well before the first real exp
            actwarm2 = pa.tile([1, 8], F32, tag="actwarm2")
            nc.scalar.activation(actwarm2, ones_t[0:1, 0:8],
                                 mybir.ActivationFunctionType.Exp)
            qan = []
            for m in range(QMT):
                qq_ = pmid.tile([128, TSH], BF, tag=f"qan{m}", name=f"qan{m}")
                nc.vector.tensor_tensor(qq_, q_raw[m], sq_bc,
                                        op=mybir.AluOpType.mult)
                qan.append(qq_)

        bcp = ctx.enter_context(tc.tile_pool(name="bcp", bufs=1))
        # kk: DoubleRow score lhsT per head: [dims, shard, half, slot, 128tok]
        # slots per token block: [kn_h0 | kpe-pair | kn_h1 | kpe-pair]
        # head h uses slots (2h, 2h+1); slot 1 == slot 3 = [kpe ; kpe-copy].
        kk = bcp.tile([128, NCORES, 2, 4, 128], F8, tag="kk", name="kk")
        kpe8 = bcp.tile([DR, NCORES, 2, 128], F8, tag="kpe8", name="kpe8")

        # ---- q_b for all dests + per-head exchange ------------------------
        # head-0 AllToAll goes first so head-0 attention can overlap the
        # head-1 AllToAll.  Exchange rows per dest: [qn128 | qpe64 | resid64].
        # The replicated-k_pe work shares these pools so nothing serializes
        # on pool open/close.
        with tc.tile_pool(name="pw", bufs=1) as pw, \
             tc.tile_pool(name="psw", bufs=1, space="PSUM") as psw:
            # hs_full strips for replicated k_pe: all on the Act queue,
            # emitted first (ready at t0; everything behind them on Act is
            # late-ready, so fetch-blocking on buffer slots is harmless)
            hstrips = []
            for qt_ in range(4):
                for k in range(KT):
                    hstrip = pw.tile([128, 512], BF, tag="hsf", bufs=14,
                                     name="hstrip")
                    nc.scalar.dma_start(
                        out=hstrip,
                        in_=hs_full[k * 128:(k + 1) * 128,
                                    qt_ * 512:(qt_ + 1) * 512])
                    hstrips.append(hstrip)
            st_qn = [pw.tile([128, NCORES, TSH], F8, tag=f"stqn{h}",
                             name=f"stqn{h}") for h in range(HPC)]
            st_pe = [pw.tile([128, NCORES, TSH], F8, tag=f"stpe{h}",
                             name=f"stpe{h}") for h in range(HPC)]
            cos64 = cos_t[0:DR, :]
            sin64 = sin_t[0:DR, :]
            for d in range(NCORES):
                wq = wq_t[d]
                accq = []
                for mt in range(3):
                    a = psw.tile([128, TSH], F32, tag="acc", bufs=4,
                                 name=f"accq{mt}")
                    accq.append(a)
                for k in range(QMT):
                    for mt in range(3):
                        nc.tensor.matmul(
                            accq[mt],
                            lhsT=wq[:, k * QCH + mt * 128:k * QCH + (mt + 1) * 128],
                            rhs=qan[k],
                            start=(k == 0), stop=(k == QMT - 1))
                for hh in range(HPC):
                    nc.vector.tensor_copy(st_qn[hh][:, d, :], accq[hh])
                # q_pe rope: the permutation is a +-32-row rotation, done as
                # partition-offset half-products on DVE (no PE matmuls)
                qraw = pw.tile([128, TSH], BF, tag="qraw", bufs=2)
                nc.vector.tensor_copy(qraw, accq[2])
                HD = DR // 2
                for hh in range(HPC):
                    base = qraw[hh * DR:(hh + 1) * DR]
                    r1 = pw.tile([DR, TSH], BF, tag=f"r1_{hh}", bufs=2)
                    nc.vector.tensor_tensor(r1, base, cos64,
                                            op=mybir.AluOpType.mult)
                    r2 = pw.tile([DR, TSH], BF, tag=f"r2_{hh}", bufs=2)
                    nc.vector.tensor_tensor(r2[0:HD, :], base[HD:DR, :],
                                            sin64[0:HD, :],
                                            op=mybir.AluOpType.mult)
                    nc.vector.tensor_tensor(r2[HD:DR, :], base[0:HD, :],
                                            sin64[HD:DR, :],
                                            op=mybir.AluOpType.mult)
                    pe_bf = pw.tile([DR, TSH], BF, tag=f"pebf{hh}", bufs=2)
                    nc.vector.tensor_tensor(pe_bf, r1, r2,
                                            op=mybir.AluOpType.add)
                    nc.vector.tensor_copy(st_pe[hh][0:DR, d, :], pe_bf)
                    # residual of the fp8 cast (mixed-dtype subtract)
                    nc.vector.tensor_tensor(st_pe[hh][DR:128, d, :],
                                            pe_bf, st_pe[hh][0:DR, d, :],
                                            op=mybir.AluOpType.subtract)
            for h in range(HPC):
                nc.sync.dma_start(
                    out=contrib_qh[h].rearrange("(d c) t -> c d t",
                                                d=NCORES)[0:128],
                    in_=st_qn[h])
                nc.sync.dma_start(
                    out=contrib_qh[h].rearrange("(d c) t -> c d t",
                                                d=NCORES)[128:QEX],
                    in_=st_pe[h])
                nc.gpsimd.collective_compute(
                    "AllToAll", mybir.AluOpType.bypass,
                    replica_groups=[list(range(NCORES))],
                    ins=[contrib_qh[h]], outs=[a2a_qh[h]])

            # ---- replicated k_pe over all tokens --------------------------
            # Real PE work that fills the AllGather window (and keeps the
            # p-state warm): k_pe = w_kpe^T . hs for all 2048 tokens, roped.
            kpel_all = pw.tile([DR, NCORES, 2, 128], BF, tag="kpelall")
            for qt_ in range(4):
                tslh = slice(qt_ * 512, (qt_ + 1) * 512)
                kpe_ps = psw.tile([DR, 512], F32, tag="kpeps", bufs=2)
                for k in range(KT):
                    nc.tensor.matmul(kpe_ps,
                                     lhsT=wkpe_t[:, k * DR:(k + 1) * DR],
                                     rhs=hstrips[qt_ * KT + k],
                                     start=(k == 0), stop=(k == KT - 1))
                kpe_rawh = pw.tile([DR, 512], BF, tag="kpraw", bufs=2)
                nc.vector.tensor_copy(kpe_rawh, kpe_ps)
                HD = DR // 2
                rt1 = pw.tile([DR, 512], BF, tag="rt1k", bufs=2)
                nc.vector.tensor_tensor(rt1, kpe_rawh, cosk_t[:, tslh],
                                        op=mybir.AluOpType.mult)
                rt2 = pw.tile([DR, 512], BF, tag="rt2k", bufs=2)
                nc.vector.tensor_tensor(rt2[0:HD, :], kpe_rawh[HD:DR, :],
                                        sink_t[0:HD, tslh],
                                        op=mybir.AluOpType.mult)
                nc.vector.tensor_tensor(rt2[HD:DR, :], kpe_rawh[0:HD, :],
                                        sink_t[HD:DR, tslh],
                                        op=mybir.AluOpType.mult)
                nc.vector.tensor_tensor(
                    kpel_all[:, 2 * qt_:2 * qt_ + 2, :, :], rt1, rt2,
                    op=mybir.AluOpType.add)
            nc.vector.tensor_copy(kpe8, kpel_all)
            for pr in range(2):
                dst = kk[pr * DR:(pr + 1) * DR]
                eng = nc.sync if pr == 0 else nc.scalar
                eng.dma_start(out=dst[:, :, :, 1, :], in_=kpe8)
                eng.dma_start(out=dst[:, :, :, 3, :], in_=kpe8)


        # ---- Phase B: expand k_nope / v for own heads over all tokens -----
        kvan = []      # latent strips, all tokens [128, 8, 256]
        for r in range(KVMT):
            kt_ = bcp.tile([128, NCORES, TSH], BF, tag=f"kvan{r}",
                           name=f"kvan{r}")
            eng = nc.sync if r % 2 == 0 else nc.scalar
            eng.dma_start(
                out=kt_,
                in_=a2a_kv.rearrange("(s r) t -> r s t", s=NCORES)
                            [r * 128:(r + 1) * 128])
            kvan.append(kt_)

        def tok512(tile3, c):
            # 512-token chunk c of a [*, 8, 256] tile
            return tile3[:, 2 * c:2 * c + 2, :]

        def tok128(tile3, tb):
            half = (tb % 2) * 128
            return tile3[:, tb // 2, half:half + 128]

        vt = [None] * TBT   # per 128-token block [128, HPC*DV] token-major v
        with tc.tile_pool(name="pb", bufs=1) as pb, \
             tc.tile_pool(name="psb", bufs=1, space="PSUM") as psb:
            for h in range(HPC):
                for c in range(4):
                    acck = psb.tile([128, 512], F32, tag="acck", bufs=2)
                    for s in range(KVMT):
                        nc.tensor.matmul(
                            acck, lhsT=wkvb_t[:, s, h * DN:(h + 1) * DN],
                            rhs=tok512(kvan[s], c),
                            start=(s == 0), stop=(s == KVMT - 1))
                    nc.vector.tensor_copy(
                        kk[:, 2 * c:2 * c + 2, :, 2 * h, :]
                        .rearrange("p s j c -> p (s j) c"),
                        acck.rearrange("p (f c) -> p f c", c=128))
            for tb in range(TBT):
                accv = psb.tile([128, HPC * DV], F32, tag="accv", bufs=3)
                for s in range(KVMT):
                    nc.tensor.matmul(
                        accv, lhsT=tok128(kvan[s], tb),
                        rhs=wkvb_t[:, s, 2 * DN:2 * DN + HPC * DV],
                        start=(s == 0), stop=(s == KVMT - 1))
                vt[tb] = bcp.tile([128, HPC * DV], BF, tag=f"v{tb}",
                                  name=f"v{tb}")
                nc.vector.tensor_copy(vt[tb], accv)

        # q tiles for own heads, all tokens: [dims, chunk, shard, 256]
        # chunk0 = qn (128 dims); chunk1 = [qpe 64 | qpe_resid 64].
        qq = []
        for h in range(HPC):
            qt = bcp.tile([128, 2, NCORES, TSH], F8, tag=f"qq{h}",
                          name=f"qq{h}")
            src = a2a_qh[h].rearrange("(s c) t -> c s t", s=NCORES)
            engs = ((nc.sync, nc.scalar) if h == 0 else (nc.gpsimd, nc.sync))
            for sh in range(2):
                ssl = slice(sh * 4, sh * 4 + 4)
                qeng = engs[sh]
                qeng.dma_start(out=qt[:, 0, ssl, :], in_=src[0:128, ssl])
                qeng.dma_start(out=qt[0:DR, 1, ssl, :],
                               in_=src[128:128 + DR, ssl])
                qeng.dma_start(out=qt[DR:128, 1, ssl, :],
                               in_=src[128 + DR:QEX, ssl])
            qq.append(qt)

        # ---- Attention + output projection --------------------------------
        with tc.tile_pool(name="pc", bufs=1) as pc, \
             tc.tile_pool(name="psc", bufs=1, space="PSUM") as psc:
            attn_n = [[None] * NB for _ in range(HPC)]
            for h in range(HPC):
                for qj in reversed(range(NB)):
                    npair = 2 * qj + 2
                    q_rhs = qq[h][:, :, 2 * qj:2 * qj + 2, :]
                    attn_ps = psc.tile([128, 512], F32, tag="attn", bufs=2)
                    esums = []   # tree-reduced pair sums (short dep chains)
                    for p in range(npair):
                        s_pair = psc.tile([128, 2, 512], F32, tag="s", bufs=2)
                        for i in range(2):
                            ki = 2 * p + i
                            diag = ki >= 4 * qj
                            nc.tensor.matmul(
                                s_pair[:, i, :],
                                lhsT=kk[:, ki // 2, ki % 2,
                                        2 * h:2 * h + 2, :],
                                rhs=q_rhs,
                                start=True, stop=not diag,
                                perf_mode=DRMODE)
                            if diag:
                                sub = ki - 4 * qj
                                nc.tensor.matmul(
                                    s_pair[:, i, :],
                                    lhsT=identz_t,
                                    rhs=maskdz_t[:, sub],
                                    start=False, stop=True,
                                    perf_mode=DRMODE)
                        e_pair = pc.tile([128, 2, 512], BF, tag="e", bufs=6)
                        nc.scalar.activation(e_pair, s_pair,
                                             mybir.ActivationFunctionType.Exp,
                                             scale=SCALE)
                        for i in range(2):
                            ki = 2 * p + i
                            nc.tensor.matmul(
                                attn_ps,
                                lhsT=vt[ki][:, h * DV:(h + 1) * DV],
                                rhs=e_pair[:, i, :],
                                start=(p == 0 and i == 0),
                                stop=(p == npair - 1 and i == 1))
                        et = pc.tile([128, 512], BF, tag="etree", bufs=8)
                        nc.vector.tensor_tensor(
                            et, e_pair[:, 0, :], e_pair[:, 1, :],
                            op=mybir.AluOpType.add)
                        esums.append(et)
                        # opportunistic tree combine of completed pairs
                        while len(esums) >= 2 and p < npair - 1:
                            b = esums.pop()
                            a = esums.pop()
                            et2 = pc.tile([128, 512], BF, tag="etree",
                                          bufs=8)
                            nc.vector.tensor_tensor(
                                et2, a, b, op=mybir.AluOpType.add)
                            esums.append(et2)
                            break
                    while len(esums) > 1:
                        b = esums.pop()
                        a = esums.pop()
                        et2 = pc.tile([128, 512], BF, tag="etree", bufs=8)
                        nc.vector.tensor_tensor(et2, a, b,
                                                op=mybir.AluOpType.add)
                        esums.append(et2)
                    esum = esums[0]
                    # z = colsum(esum); broadcast 1/z via ones matmul
                    zrow_ps = psc.tile([128, 512], F32, tag="o", bufs=2,
                                       name="zrow_ps")
                    nc.tensor.matmul(zrow_ps[0:1, :], lhsT=ones_col,
                                     rhs=esum, start=True, stop=True)
                    zr = pc.tile([1, 512], BF, tag="zr", bufs=2)
                    with nc.allow_low_precision(reason="bf16 softmax"):
                        nc.vector.tensor_copy(zr, zrow_ps[0:1, :])
                    zb_ps = psc.tile([128, 512], F32, tag="o", bufs=2,
                                     name="zb_ps")
                    nc.tensor.matmul(zb_ps, lhsT=ones_row, rhs=zr,
                                     start=True, stop=True)
                    rzb = pc.tile([128, 512], BF, tag="rzb", bufs=2)
                    with nc.allow_low_precision(reason="bf16 softmax"):
                        nc.vector.reciprocal(rzb, zb_ps)
                    attn_n[h][qj] = bcp.tile([128, 512], BF,
                                             tag=f"attn{h}_{qj}",
                                             name=f"attn{h}_{qj}")
                    nc.vector.tensor_tensor(attn_n[h][qj], attn_ps, rzb,
                                            op=mybir.AluOpType.mult)

                    if h == HPC - 1:
                        # both heads' attn_n for this query block are ready
                        for tt in range(4):
                            tb = qj * 4 + tt
                            tsl = slice(tt * 128, (tt + 1) * 128)
                            last = (qj == 0 and tt == 3)  # last in exec order
                            o_row = pc.tile([128, HID], BF, tag="orow",
                                            bufs=2)
                            for hb in range(NB):
                                o_ps = psc.tile([128, 512], F32, tag="o",
                                                bufs=2)
                                for hh in range(HPC):
                                    nc.tensor.matmul(
                                        o_ps,
                                        lhsT=attn_n[hh][qj][:, tsl],
                                        rhs=wo_t[hh][:, hb * 512:(hb + 1) * 512],
                                        start=(hh == 0),
                                        stop=(hh == HPC - 1),
                                    )
                                osl = o_row[:, hb * 512:(hb + 1) * 512]
                                if hb % 2 == 0:
                                    nc.vector.tensor_copy(osl, o_ps)
                                else:
                                    nc.gpsimd.tensor_copy(osl, o_ps)
                                if last:
                                    weng = nc.sync if hb % 2 == 0 else nc.scalar
                                    weng.dma_start(
                                        out=out[tb * 128:(tb + 1) * 128,
                                                hb * 512:(hb + 1) * 512],
                                        in_=osl)
                            if not last:
                                nc.sync.dma_start(
                                    out=out[tb * 128:(tb + 1) * 128, :],
                                    in_=o_row)


_NC_CACHE = {}


def _get_nc():
    if "nc" not in _NC_CACHE:
        _NC_CACHE["nc"] = build_bass()
    return _NC_CACHE["nc"]


def make_in_maps(positions, hidden_states, w_q_a, q_a_ln_w, w_q_b, w_kv_a,
                 kv_a_ln_w, w_kv_b, w_o):
    positions = np.asarray(positions)
    hidden_states = np.asarray(hidden_states, dtype=np.float32)
    w_q_a = np.asarray(w_q_a, dtype=np.float32)
    q_a_ln_w = np.asarray(q_a_ln_w, dtype=np.float32)
    w_q_b = np.asarray(w_q_b, dtype=np.float32)
    w_kv_a = np.asarray(w_kv_a, dtype=np.float32)
    kv_a_ln_w = np.asarray(kv_a_ln_w, dtype=np.float32)
    w_kv_b = np.asarray(w_kv_b, dtype=np.float32)
    w_o = np.asarray(w_o, dtype=np.float32)

    hs_t = np.ascontiguousarray(hidden_states.T)

    order = np.concatenate([np.arange(0, DR, 2), np.arange(1, DR, 2)])

    wkva_p = w_kv_a.copy()
    wkva_p[:, KVLR:] = w_kv_a[:, KVLR:][:, order]

    inv_freq = 1.0 / (THETA ** (np.arange(0, DR, 2, dtype=np.float64) / DR))
    ang = positions.astype(np.float64)[:, None] * inv_freq[None, :]
    cosT = np.cos(ang).T.astype(np.float32)
    sinT = np.sin(ang).T.astype(np.float32)
    cosf = np.concatenate([cosT, cosT], axis=0)          # [64, T]
    sinf = np.concatenate([-sinT, sinT], axis=0)
    cosf2 = np.concatenate([cosf, cosf], axis=0)         # [128, T] two heads
    sinf2 = np.concatenate([sinf, sinf], axis=0)

    perm = np.zeros((DR, DR), dtype=np.float32)
    for i in range(DR):
        perm[i, (i + DR // 2) % DR] = 1.0
    perm128 = np.zeros((128, 128), dtype=np.float32)
    perm128[:DR, :DR] = perm
    perm128[DR:, DR:] = perm
    selswap = np.zeros((128, 128), dtype=np.float32)
    for i in range(DR):
        selswap[DR + i, i] = 1.0                      # extract h1 raw
        selswap[DR + (i + DR // 2) % DR, DR + i] = 1.0  # extract h1 swapped

    # DoubleRow mask operands: chunk0 carries the additive causal mask for
    # the 4 diagonal sub-positions ({0, -448}, exact in fp8e4m3), chunk1 = 0.
    identz = np.zeros((128, 2, 128), dtype=np.float32)
    identz[:, 0, :] = np.eye(128, dtype=np.float32)
    maskdz = np.zeros((128, 4, 2, 512), dtype=np.float32)
    p = np.arange(128)[:, None]
    f = np.arange(512)[None, :]
    for sub in range(4):
        maskdz[:, sub, 0, :] = np.where(p + 128 * sub <= f, 0.0, MASKV)

    # q_b columns per dest: [qn_h0 | qn_h1 | qpe_h0(perm) ; qpe_h1(perm)]
    # NOTE: no SCALE fold — softmax scale is applied inside the exp.
    wqb_all = np.concatenate([
        np.concatenate([
            w_q_b[:, h0 * DQK:h0 * DQK + DN],
            w_q_b[:, h1 * DQK:h1 * DQK + DN],
            w_q_b[:, h0 * DQK + DN:(h0 + 1) * DQK][:, order],
            w_q_b[:, h1 * DQK + DN:(h1 + 1) * DQK][:, order],
        ], axis=1)
        for h0, h1 in ((2 * d, 2 * d + 1) for d in range(NCORES))
    ], axis=1) * q_a_ln_w[:, None]

    def pack(w, mrows):
        Kd, Md = w.shape
        n = Md // mrows
        return np.ascontiguousarray(
            w.reshape(Kd // 128, 128, n, mrows).transpose(2, 1, 0, 3)
            .reshape(n * 128, (Kd // 128) * mrows))

    wqa_pk = pack(w_q_a, 128)
    wkva_pk = pack(wkva_p[:, :KVLR], 128)
    wkpe_pk = pack(wkva_p[:, KVLR:], DR)
    wqb_pk = pack(wqb_all, QCH)

    def bf(x):
        return np.ascontiguousarray(np.asarray(x, dtype=np.float32)).astype(BF_NP)

    def f8(x):
        return np.ascontiguousarray(np.asarray(x, dtype=np.float32)).astype(F8_NP)

    in_maps = []
    for c in range(NCORES):
        h0, h1 = HPC * c, HPC * c + 1
        # own-head kv_b columns: [kn_h0 | kn_h1 | v_h0 | v_h1], ln folded
        wkvb_own = np.concatenate([
            w_kv_b[:, h0 * (DN + DV):h0 * (DN + DV) + DN],
            w_kv_b[:, h1 * (DN + DV):h1 * (DN + DV) + DN],
            w_kv_b[:, h0 * (DN + DV) + DN:(h0 + 1) * (DN + DV)],
            w_kv_b[:, h1 * (DN + DV) + DN:(h1 + 1) * (DN + DV)],
        ], axis=1) * kv_a_ln_w[:, None]
        wkvb_pk = pack(wkvb_own, 4 * 128)
        wo_c = np.concatenate([
            w_o[h0 * DV:(h0 + 1) * DV, :],
            w_o[h1 * DV:(h1 + 1) * DV, :],
        ], axis=0)
        tsl = slice(c * TSH, (c + 1) * TSH)
        in_maps.append({
            "hs_sh": bf(hs_t[:, tsl]),
            "hs_full": bf(hs_t),
            "cosk": bf(cosf),
            "sink": bf(sinf),
            "wqa": bf(wqa_pk),
            "wkva": bf(wkva_pk),
            "wkpe": bf(wkpe_pk),
            "wqb": bf(wqb_pk),
            "wkvb": bf(wkvb_pk),
            "wo": bf(wo_c),
            "cosf2": bf(cosf2[:, tsl]),
            "sinf2": bf(sinf2[:, tsl]),
            "perm128": bf(perm128),
            "selswap": bf(selswap),
            "identz": f8(identz.reshape(128, 2 * 128)),
            "maskdz": f8(maskdz.reshape(128, 4 * 1024)),
            "ones": bf(np.ones((128, 128), dtype=np.float32)),
        })
    return in_maps


def kernel(positions, hidden_states, w_q_a, q_a_ln_w, w_q_b, w_kv_a,
           kv_a_ln_w, w_kv_b, w_o):
    nc = _get_nc()
    in_maps = make_in_maps(positions, hidden_states, w_q_a, q_a_ln_w, w_q_b,
                           w_kv_a, kv_a_ln_w, w_kv_b, w_o)
    res = bass_utils.run_bass_kernel_spmd(nc, in_maps, core_ids=list(range(NCORES)))
    acc = np.zeros((T, HID), dtype=np.float32)
    for c in range(NCORES):
        acc += np.asarray(res.results[c]["out"], dtype=np.float32)
    return acc
